# revision 13
# baseline (speedup 1.0000x reference)
"""GAU (Gated Attention Unit, relu^2 attention) Trainium2 Bass kernel, fp8.

Problem shapes: x [4, 2048, 2048] f32; W_hidden [2048, 8192]; W_qk [2048, 128];
W_out [4096, 2048]; out = GAU(x) + x.

Sharding (8 cores): core = 2*batch + h, h in {0,1}.  Each pair of cores
handles one batch.  The VALUE/GATE hidden projections are column-split in
half within the pair (core h computes v for hidden cols
[h*2048:(h+1)*2048]); the computed v half is then exchanged with a
pairwise AllGather (fp8, 4MB, pipelined per 512-column slab behind the
gate projection) so both cores hold the full [2048, 4096] v.  An
AllGather works here precisely because both cores want the SAME full v
-- its output is indexed by source rank, which equals the hidden-half
index, keeping the program rank-independent.

Everything downstream is split by QUERY rows instead: core h owns the
global rows [h*1024, (h+1)*1024) and computes scores, attn, gate, og and
the output projection (full 4096-deep contraction, W_out streamed fp8)
only for those rows, writing final bf16 branch rows straight to its
output.  The row split is baked in on the host: a second input tensor
x8my carries only the core's own 1024 x columns for the q projection and
the gate's moving operand, while x8g (global order) feeds k, and v.
There is no partial-sum ReduceScatter and no serial collective tail.
The host adds the residual x and b_out in fp32 (the branch is ~0.3% of
the output norm, so a bf16 branch costs ~3e-6 relative error).

All projections run as fp8e4 (e4m3, max +-240) DoubleRow matmuls: both
operands are packed [128, 2, free] pairing two 128-row contraction
tiles, giving 2 fp8 MACs/PE-cell/cycle.  Quantization scales are powers
of two, chosen so every fp8 tensor's max is >=2.7x below 240 (measured
on the real data distribution; host clips before casting to avoid
e4m3's non-saturating inf at 256):

  x*16, W_hidden*512, W_qk*512, W_out*512, v*16, attn*2^14, gate*32,
  og*1024

PSUM stays fp32; descales fold into the activation `scale` ports and DVE
tensor_scalar immediates.  qT/kT and the relu^2 scores matmul stay bf16
(K=128, cheap; scores are NOT pair-replicated since each core only needs
its own rows).
"""

import numpy as np
import ml_dtypes
from contextlib import ExitStack

import concourse.bass as bass
import concourse.bacc as bacc
import concourse.mybir as mybir
import concourse.tile as tile
from concourse.bass_utils import run_bass_kernel_spmd

BF16 = mybir.dt.bfloat16
F32 = mybir.dt.float32
F8 = mybir.dt.float8e4
AF = mybir.ActivationFunctionType
ALU = mybir.AluOpType
DR = mybir.MatmulPerfMode.DoubleRow
P = 128
F8MAX = 240.0

# quantization scales (powers of two; see module docstring)
SX = 16.0       # x
SW = 512.0      # W_hidden (both halves), W_qk
SWO = 512.0     # W_out
SV = 16.0       # v activations
SA = 16384.0    # attn = relu(sim)^2    (sqrt(SA) = 128 folds into the relu)
KGF = 32.0      # gate activations
SO = 1024.0     # og activations

INV_H = 1.0 / (SX * SW)        # hidden/qk psum -> real pre-activation
T_V = SV / (SX * SW)           # v-path u multiplier
T_G = KGF / (SX * SW)          # gate-path u multiplier
OG_SC = SO / (SA * SV * KGF)   # og = (psum_av * OG_SC) * gate8
RSC = 128.0 / 2048.0           # sqrt(SA)/seq relu scale
INV_O = 1.0 / (SO * SWO)       # out psum -> real branch rows


def build_gau_nc(seq=2048, dim=2048, hh=2048, n_cores=8, with_bhv=True):
    """Build the SPMD program.  hh = per-core v half width."""
    e = P
    nd2 = dim // (2 * P)   # DoubleRow contraction pair tiles
    njt = seq // P
    njp = njt // 2
    H = 2 * hh             # full hidden width
    nct = H // P           # full-hidden c tiles (gate, og)
    ncp = nct // 2
    nmy = seq // 2         # own query rows per core
    IC = 512
    n_ic = seq // IC
    n_mc = nmy // IC       # own-row chunks
    CC = 512
    n_cc = hh // CC
    DC = 512
    n_dc = dim // DC
    pairs = [[2 * g, 2 * g + 1] for g in range(n_cores // 2)]

    nc = bacc.Bacc("TRN2", target_bir_lowering=False, debug=False,
                   num_devices=n_cores)

    x8g_d = nc.dram_tensor("x8g", [P, nd2, 2, seq], F8, kind="ExternalInput")
    x8my_d = nc.dram_tensor("x8my", [P, nd2, 2, nmy], F8,
                            kind="ExternalInput")
    whv8_d = nc.dram_tensor("whv8", [P, nd2, 2, hh], F8, kind="ExternalInput")
    whg8_d = nc.dram_tensor("whg8", [P, nd2, 2, H], F8, kind="ExternalInput")
    wqk8_d = nc.dram_tensor("wqk8", [P, nd2, 2, e], F8, kind="ExternalInput")
    wout8_d = nc.dram_tensor("wout8", [P, ncp, 2, dim], F8,
                             kind="ExternalInput")
    bqk_d = nc.dram_tensor("bqk", [e, 1], F32, kind="ExternalInput")
    bqks_d = nc.dram_tensor("bqks", [e, 1], F32, kind="ExternalInput")
    gq_d = nc.dram_tensor("gq", [e, 1], F32, kind="ExternalInput")
    bq_d = nc.dram_tensor("bq", [e, 1], F32, kind="ExternalInput")
    gk_d = nc.dram_tensor("gk", [e, 1], F32, kind="ExternalInput")
    bk_d = nc.dram_tensor("bk", [e, 1], F32, kind="ExternalInput")
    bhv_d = nc.dram_tensor("bhv", [1, hh], F32, kind="ExternalInput")
    bhgT_d = nc.dram_tensor("bhgT", [P, nct], F32, kind="ExternalInput")
    bhgTs_d = nc.dram_tensor("bhgTs", [P, nct], F32, kind="ExternalInput")
    out_d = nc.dram_tensor("out", [nmy, dim], BF16, kind="ExternalOutput")

    with tile.TileContext(nc) as tc, ExitStack() as st:
        constp = st.enter_context(tc.tile_pool(name="const", bufs=1))
        psump = st.enter_context(tc.tile_pool(name="psum", bufs=8, space="PSUM"))
        dramp = st.enter_context(tc.tile_pool(name="dram", bufs=1, space="DRAM"))
        mainp = st.enter_context(tc.tile_pool(name="main", bufs=1))

        # v AllGather bounce buffers, one per 512-col slab of the own half
        vgi = [dramp.tile([P, njt, CC], F8, tag=f"vgi{k}", name=f"vgi{k}")
               for k in range(n_cc)]
        vgo = [dramp.tile([2, P, njt, CC], F8, tag=f"vgo{k}", name=f"vgo{k}")
               for k in range(n_cc)]

        # ---- constants ----
        wqk8_sb = constp.tile([P, nd2, 2, e], F8, tag="wqk8")
        nc.sync.dma_start(wqk8_sb[:], wqk8_d[:])
        bqk_sb = constp.tile([e, 1], F32, tag="bqk")
        nc.sync.dma_start(bqk_sb[:], bqk_d[:])
        bqks_sb = constp.tile([e, 1], F32, tag="bqks")
        nc.sync.dma_start(bqks_sb[:], bqks_d[:])
        gq_sb = constp.tile([e, 1], F32, tag="gq")
        nc.sync.dma_start(gq_sb[:], gq_d[:])
        bq_sb = constp.tile([e, 1], F32, tag="bq")
        nc.sync.dma_start(bq_sb[:], bq_d[:])
        gk_sb = constp.tile([e, 1], F32, tag="gk")
        nc.sync.dma_start(gk_sb[:], gk_d[:])
        bk_sb = constp.tile([e, 1], F32, tag="bk")
        nc.sync.dma_start(bk_sb[:], bk_d[:])
        bhg_sb = constp.tile([P, nct], F32, tag="bhg")
        nc.sync.dma_start(bhg_sb[:], bhgT_d[:])
        bhgs_sb = constp.tile([P, nct], F32, tag="bhgs")
        nc.sync.dma_start(bhgs_sb[:], bhgTs_d[:])
        bhv_sb = constp.tile([1, hh], F32, tag="bhv")
        nc.sync.dma_start(bhv_sb[:], bhv_d[:])
        ones_sb = constp.tile([1, P], F32, tag="ones")
        nc.vector.memset(ones_sb[:], 1.0)

        # tiny AllGather to warm the collective stream (first real op
        # otherwise pays a ~50us cold-start)
        warm_in = dramp.tile([1, 64], F32, tag="warm_in", name="warm_in")
        warm_out = dramp.tile([2, 64], F32, tag="warm_out", name="warm_out")
        warm_sb = constp.tile([1, 64], F32, tag="warm_sb")
        nc.vector.memset(warm_sb[:], 0.0)
        nc.gpsimd.dma_start(warm_in[:], warm_sb[:])
        nc.gpsimd.collective_compute("AllGather", ALU.bypass,
                                     replica_groups=pairs,
                                     ins=[warm_in.opt()],
                                     outs=[warm_out.opt()])

        # persistent activations
        qT_sb = mainp.tile([e, nmy], BF16, tag="qT", name="qT")
        kT_sb = mainp.tile([e, seq], BF16, tag="kT", name="kT")
        v8_sb = mainp.tile([P, njt, H], F8, tag="v8", name="v8")
        gate8_sb = mainp.tile([P, nct, nmy], F8, tag="gate8", name="gate8")

        with tc.tile_pool(name="ph1", bufs=1) as ph1p, \
             tc.tile_pool(name="wstream", bufs=1) as wsp:
            x8g_sb = ph1p.tile([P, nd2, 2, seq], F8, tag="x8g", name="x8g")
            for quar in range(4):
                cols = slice(quar * (seq // 4), (quar + 1) * (seq // 4))
                nc.sync.dma_start(x8g_sb[:, :, :, cols], x8g_d[:, :, :, cols])
            x8m_sb = ph1p.tile([P, nd2, 2, nmy], F8, tag="x8m", name="x8m")
            nc.sync.dma_start(x8m_sb[:], x8my_d[:])

            # ---- qk projection (fp8 DoubleRow) ----
            # kT from the global-order x, qT from the own-rows x.
            # silu(u) = u * sigmoid(u); no Silu LUT in the interp, so build
            # from Sigmoid (ACT) + mult (DVE).
            with tc.tile_pool(name="qkp", bufs=1) as qkp:
                qkg_sb = qkp.tile([e, seq], BF16, tag="qkg", name="qkg")
                qkm_sb = qkp.tile([e, nmy], BF16, tag="qkm", name="qkm")
                for src, n_chunks, xsb, qk_sb in (
                        (0, n_ic, x8g_sb, qkg_sb), (1, n_mc, x8m_sb, qkm_sb)):
                    for ic in range(n_chunks):
                        cols = slice(ic * IC, (ic + 1) * IC)
                        ps = psump.tile([e, IC], F32, tag="ps", name="ps")
                        for dp in range(nd2):
                            nc.tensor.matmul(ps[:], wqk8_sb[:, dp],
                                             xsb[:, dp, :, cols],
                                             start=(dp == 0),
                                             stop=(dp == nd2 - 1),
                                             perf_mode=DR)
                        sg = qkp.tile([e, IC], BF16, tag="sg1", bufs=2,
                                      name="sg")
                        nc.scalar.activation(sg[:], ps[:], AF.Sigmoid,
                                             bias=bqk_sb[:], scale=INV_H)
                        u = qkp.tile([e, IC], BF16, tag="u1", bufs=2, name="u")
                        nc.vector.tensor_scalar(u[:], ps[:], bqks_sb[:],
                                                INV_H, ALU.add, ALU.mult)
                        nc.vector.tensor_tensor(qk_sb[:, cols], u[:], sg[:],
                                                ALU.mult)
                nc.vector.tensor_scalar(kT_sb[:], qkg_sb[:], gk_sb[:],
                                        bk_sb[:], ALU.mult, ALU.add)
                nc.vector.tensor_scalar(qT_sb[:], qkm_sb[:], gq_sb[:],
                                        bq_sb[:], ALU.mult, ALU.add)

            # ---- v half: v[j, c], exchanged via pairwise AllGather ----
            # the AG output is indexed by source rank = hidden-half index,
            # so assembling the full v is rank-independent; the per-slab
            # pipeline hides the exchange behind the gate projection below
            for cc in range(n_cc):
                ccs = slice(cc * CC, (cc + 1) * CC)
                wv8 = [wsp.tile([P, 2, CC], F8, tag=f"wv{dp}", bufs=2,
                                name=f"wv{dp}") for dp in range(nd2)]
                for dp in range(nd2):
                    nc.sync.dma_start(wv8[dp][:], whv8_d[:, dp, :, ccs])
                for jt in range(njt):
                    jts = slice(jt * P, (jt + 1) * P)
                    ps = psump.tile([P, CC], F32, tag="ps", name="ps")
                    for dp in range(nd2):
                        nc.tensor.matmul(ps[:], x8g_sb[:, dp, :, jts],
                                         wv8[dp][:], start=(dp == 0),
                                         stop=(not with_bhv and dp == nd2 - 1),
                                         perf_mode=DR)
                    if with_bhv:
                        nc.tensor.matmul(ps[:], ones_sb[:], bhv_sb[:, ccs],
                                         start=False, stop=True)
                    sg = wsp.tile([P, CC], BF16, tag="sgv", bufs=2, name="sgv")
                    nc.scalar.activation(sg[:], ps[:], AF.Sigmoid, scale=INV_H)
                    t = wsp.tile([P, CC], BF16, tag="tv", bufs=2, name="tv")
                    nc.vector.tensor_scalar(t[:], ps[:], T_V, None, ALU.mult)
                    vst = wsp.tile([P, CC], F8, tag="vst", bufs=3, name="vst")
                    nc.vector.tensor_tensor(vst[:], t[:], sg[:], ALU.mult)
                    nc.scalar.dma_start(vgi[cc][:, jt, :], vst[:])
                nc.gpsimd.collective_compute("AllGather", ALU.bypass,
                                             replica_groups=pairs,
                                             ins=[vgi[cc].opt()],
                                             outs=[vgo[cc].opt()])
                for s in range(2):
                    c0 = s * hh + cc * CC
                    nc.gpsimd.dma_start(v8_sb[:, :, c0:c0 + CC], vgo[cc][s])

            # ---- gate: gateT[c, i] over the FULL hidden, own rows only ----
            for ct in range(nct):
                cts = slice(ct * P, (ct + 1) * P)
                wg8 = [wsp.tile([P, 2, P], F8, tag=f"wg{dp}", bufs=1,
                                name=f"wg{dp}") for dp in range(nd2)]
                for dp in range(nd2):
                    nc.sync.dma_start(wg8[dp][:], whg8_d[:, dp, :, cts])
                for ic in range(n_mc):
                    cols = slice(ic * IC, (ic + 1) * IC)
                    ps = psump.tile([P, IC], F32, tag="ps", name="ps")
                    for dp in range(nd2):
                        nc.tensor.matmul(ps[:], wg8[dp][:],
                                         x8m_sb[:, dp, :, cols],
                                         start=(dp == 0), stop=(dp == nd2 - 1),
                                         perf_mode=DR)
                    sg = wsp.tile([P, IC], BF16, tag="sgg", bufs=2, name="sgg")
                    nc.scalar.activation(sg[:], ps[:], AF.Sigmoid,
                                         bias=bhg_sb[:, ct:ct + 1],
                                         scale=INV_H)
                    t = wsp.tile([P, IC], BF16, tag="tg", bufs=2, name="tg")
                    nc.vector.tensor_scalar(t[:], ps[:],
                                            bhgs_sb[:, ct:ct + 1], T_G,
                                            ALU.add, ALU.mult)
                    nc.vector.tensor_tensor(gate8_sb[:, ct, cols], t[:],
                                            sg[:], ALU.mult)

        # ---- attention + output over own rows ----
        with tc.tile_pool(name="ph2", bufs=1) as ph2p:
            og8_sb = ph2p.tile([P, nct, nmy], F8, tag="og8", name="og8")
            for c in range(n_mc):
                chunk = slice(c * IC, (c + 1) * IC)
                # attnT[j, own chunk] = relu(sim*sqrt(sa)/seq)^2 -> fp8
                at8 = ph2p.tile([P, njt, IC], F8, tag="at8", name="at8")
                for jt in range(njt):
                    ps = psump.tile([P, IC], F32, tag="ps", name="ps")
                    nc.tensor.matmul(ps[:], kT_sb[:, jt * P:(jt + 1) * P],
                                     qT_sb[:, chunk], start=True, stop=True)
                    rstage = ph2p.tile([P, IC], BF16, tag="rstage", bufs=4,
                                       name="rstage")
                    nc.scalar.activation(rstage[:], ps[:], AF.Relu, scale=RSC)
                    nc.vector.tensor_tensor(at8[:, jt, :], rstage[:],
                                            rstage[:], ALU.mult)
                # ogT[c over FULL hidden, own chunk] = (v^T attnT) * gate
                for ct in range(nct):
                    cts = slice(ct * P, (ct + 1) * P)
                    ps = psump.tile([P, IC], F32, tag="ps", name="ps")
                    for jp in range(njp):
                        nc.tensor.matmul(ps[:],
                                         v8_sb[:, 2 * jp:2 * jp + 2, cts],
                                         at8[:, 2 * jp:2 * jp + 2, :],
                                         start=(jp == 0), stop=(jp == njp - 1),
                                         perf_mode=DR)
                    t2 = ph2p.tile([P, IC], BF16, tag="t2", bufs=2, name="t2")
                    nc.vector.tensor_scalar(t2[:], ps[:], OG_SC, None,
                                            ALU.mult)
                    nc.vector.tensor_tensor(og8_sb[:, ct, chunk], t2[:],
                                            gate8_sb[:, ct, chunk], ALU.mult)
            # branch[own rows, :] = ogT^T @ Wout (full 4096 contraction),
            # fp8 W_out streamed per 512-col slab, straight to the output
            for dc in range(n_dc):
                dcs = slice(dc * DC, (dc + 1) * DC)
                wo8 = ph2p.tile([P, ncp, 2, DC], F8, tag="wo8", bufs=2,
                                name="wo8")
                nc.sync.dma_start(wo8[:], wout8_d[:, :, :, dcs])
                for it in range(nmy // P):
                    its = slice(it * P, (it + 1) * P)
                    ps = psump.tile([P, DC], F32, tag="ps", name="ps")
                    for cp in range(ncp):
                        nc.tensor.matmul(ps[:],
                                         og8_sb[:, 2 * cp:2 * cp + 2, its],
                                         wo8[:, cp, :, :],
                                         start=(cp == 0),
                                         stop=(cp == ncp - 1),
                                         perf_mode=DR)
                    po = ph2p.tile([P, DC], BF16, tag="po", bufs=4, name="po")
                    nc.vector.tensor_scalar(po[:], ps[:], INV_O, None,
                                            ALU.mult)
                    nc.scalar.dma_start(out_d[it * P:(it + 1) * P, dcs],
                                        po[:])

    nc.compile()
    return nc


def own_rows(seq, h):
    """Rows owned by pair-member h: contiguous half of the sequence."""
    nmy = seq // 2
    return np.arange(h * nmy, (h + 1) * nmy)


def _q8(a, s):
    """Quantize a*s to fp8 e4m3, clipping to +-240 (TRN e4m3 infs at 256)."""
    return np.clip(a.astype(np.float32) * s, -F8MAX, F8MAX).astype(
        ml_dtypes.float8_e4m3)


def _dr_pack(m8):
    """[K, F] fp8 -> [128, K//256, 2, F] DoubleRow pair layout."""
    K, F = m8.shape
    return np.ascontiguousarray(
        m8.reshape(K // 256, 2, P, F).transpose(2, 0, 1, 3))


def make_in_maps(x, W_hidden, b_hidden, W_qk, b_qk, gamma_q, beta_q,
                 gamma_k, beta_k, W_out, b_out, n_cores=8):
    """Host-side quantization/layout prep.  Returns per-core input dicts."""
    B, seq, dim = x.shape
    H2 = W_hidden.shape[1]
    H = H2 // 2
    hh = H // 2
    nct = H // P
    nmy = seq // 2
    f32 = np.float32
    x8_cache = {}
    xm_cache = {}
    half_cache = {}
    # full-hidden tensors, shared by both pair members
    whg8 = _dr_pack(_q8(W_hidden[:, H:], SW))
    wout8 = _dr_pack(_q8(W_out, SWO))
    wqk8 = _dr_pack(_q8(W_qk, SW))
    bhgT = np.ascontiguousarray(b_hidden[H:].astype(f32).reshape(nct, P).T)

    def halves(h):
        if h not in half_cache:
            cs = slice(h * hh, (h + 1) * hh)
            half_cache[h] = {
                "whv8": _dr_pack(_q8(W_hidden[:, cs], SW)),
                "bhv": (b_hidden[cs].astype(f32) * (SX * SW)).reshape(1, -1),
            }
        return half_cache[h]

    in_maps = []
    for core in range(n_cores):
        b, h = core // 2, core % 2
        if b not in x8_cache:
            x8_cache[b] = _dr_pack(_q8(np.ascontiguousarray(x[b].T), SX))
        if (b, h) not in xm_cache:
            xm_cache[(b, h)] = np.ascontiguousarray(
                x8_cache[b][:, :, :, h * nmy:(h + 1) * nmy])
        hv = halves(h)
        in_maps.append({
            "x8g": x8_cache[b],
            "x8my": xm_cache[(b, h)],
            "whv8": hv["whv8"],
            "whg8": whg8,
            "wout8": wout8,
            "wqk8": wqk8,
            "bqk": b_qk.reshape(-1, 1).astype(f32),
            "bqks": (b_qk.reshape(-1, 1).astype(f32) * (SX * SW)),
            "gq": gamma_q.reshape(-1, 1).astype(f32),
            "bq": beta_q.reshape(-1, 1).astype(f32),
            "gk": gamma_k.reshape(-1, 1).astype(f32),
            "bk": beta_k.reshape(-1, 1).astype(f32),
            "bhv": hv["bhv"],
            "bhgT": bhgT,
            "bhgTs": bhgT * (SX * SW),
        })
    return in_maps


_NC_CACHE = {}


def _get_nc(seq, dim, hh, n_cores, with_bhv=True):
    key = (seq, dim, hh, n_cores, with_bhv)
    if key not in _NC_CACHE:
        _NC_CACHE[key] = build_gau_nc(seq=seq, dim=dim, hh=hh,
                                      n_cores=n_cores, with_bhv=with_bhv)
    return _NC_CACHE[key]


def kernel(x, W_hidden, b_hidden, W_qk, b_qk, gamma_q, beta_q, gamma_k,
           beta_k, W_out, b_out):
    x = np.asarray(x)
    B, seq, dim = x.shape
    hh = W_hidden.shape[1] // 4
    n_cores = 2 * B
    with_bhv = bool(np.any(np.asarray(b_hidden)[: 2 * hh] != 0))
    nc = _get_nc(seq, dim, hh, n_cores, with_bhv=with_bhv)
    in_maps = make_in_maps(x, np.asarray(W_hidden), np.asarray(b_hidden),
                           np.asarray(W_qk), np.asarray(b_qk),
                           np.asarray(gamma_q), np.asarray(beta_q),
                           np.asarray(gamma_k), np.asarray(beta_k),
                           np.asarray(W_out), np.asarray(b_out),
                           n_cores=n_cores)
    res = run_bass_kernel_spmd(nc, in_maps, core_ids=list(range(n_cores)))
    bo = np.asarray(b_out).astype(np.float32)
    out = np.empty((B, seq, dim), np.float32)
    for b in range(B):
        for h in range(2):
            rows = own_rows(seq, h)
            out[b, rows] = (res.results[2 * b + h]["out"].astype(np.float32)
                            + x[b][rows] + bo)
    return out


# revision 14
# speedup vs baseline: 1.0354x; 1.0354x over previous
"""GAU (Gated Attention Unit, relu^2 attention) Trainium2 Bass kernel, fp8.

Problem shapes: x [4, 2048, 2048] f32; W_hidden [2048, 8192]; W_qk [2048, 128];
W_out [4096, 2048]; out = GAU(x) + x.

Sharding (8 cores): core = 2*batch + h, h in {0,1}.  Each pair of cores
handles one batch.  The VALUE/GATE hidden projections are column-split in
half within the pair (core h computes v for hidden cols
[h*2048:(h+1)*2048]); the computed v half is then exchanged with a
pairwise AllGather (fp8, 4MB, pipelined per 512-column slab behind the
gate projection) so both cores hold the full [2048, 4096] v.  An
AllGather works here precisely because both cores want the SAME full v
-- its output is indexed by source rank, which equals the hidden-half
index, keeping the program rank-independent.

Everything downstream is split by QUERY rows instead: core h owns the
global rows [h*1024, (h+1)*1024) and computes scores, attn, gate, og and
the output projection (full 4096-deep contraction, W_out streamed fp8)
only for those rows, writing final bf16 branch rows straight to its
output.  The row split is baked in on the host: a second input tensor
x8my carries only the core's own 1024 x columns for the q projection and
the gate's moving operand, while x8g (global order) feeds k, and v.
There is no partial-sum ReduceScatter and no serial collective tail.
The host adds the residual x and b_out in fp32 (the branch is ~0.3% of
the output norm, so a bf16 branch costs ~3e-6 relative error).

All projections run as fp8e4 (e4m3, max +-240) DoubleRow matmuls: both
operands are packed [128, 2, free] pairing two 128-row contraction
tiles, giving 2 fp8 MACs/PE-cell/cycle.  Quantization scales are powers
of two, chosen so every fp8 tensor's max is >=2.7x below 240 (measured
on the real data distribution; host clips before casting to avoid
e4m3's non-saturating inf at 256):

  x*16, W_hidden*512, W_qk*512, W_out*512, v*16, attn*2^14, gate*32,
  og*1024

PSUM stays fp32; descales fold into the activation `scale` ports and DVE
tensor_scalar immediates.  silu runs on the scalar engine's native Silu
LUT (one ACT op; the CoreSim interp lacks the LUT, so this kernel is
hardware-only).  qT/kT and the relu^2 scores matmul stay bf16 (K=128,
cheap; scores are NOT pair-replicated since each core only needs its own
rows).  The scores for chunk 0 are emitted between v and gate, and for
chunk 1 between og(0) and og(1), hiding their scalar-engine relu under
matmul-heavy phases.  W_out streams per 512-column output slab from a
slab-contiguous DRAM layout, double-buffered two slabs ahead.
"""

import numpy as np
import ml_dtypes
from contextlib import ExitStack

import concourse.bass as bass
import concourse.bacc as bacc
import concourse.mybir as mybir
import concourse.tile as tile
from concourse.bass_utils import run_bass_kernel_spmd

BF16 = mybir.dt.bfloat16
F32 = mybir.dt.float32
F8 = mybir.dt.float8e4
AF = mybir.ActivationFunctionType
ALU = mybir.AluOpType
DR = mybir.MatmulPerfMode.DoubleRow
P = 128
F8MAX = 240.0

# quantization scales (powers of two; see module docstring)
SX = 16.0       # x
SW = 512.0      # W_hidden (both halves), W_qk
SWO = 512.0     # W_out
SV = 16.0       # v activations
SA = 16384.0    # attn = relu(sim)^2    (sqrt(SA) = 128 folds into the relu)
KGF = 32.0      # gate activations
SO = 1024.0     # og activations

INV_H = 1.0 / (SX * SW)        # hidden/qk psum -> real pre-activation
T_V = SV / (SX * SW)           # v-path u multiplier
T_G = KGF / (SX * SW)          # gate-path u multiplier
OG_SC = SO / (SA * SV * KGF)   # og = (psum_av * OG_SC) * gate8
RSC = 128.0 / 2048.0           # sqrt(SA)/seq relu scale
INV_O = 1.0 / (SO * SWO)       # out psum -> real branch rows


def build_gau_nc(seq=2048, dim=2048, hh=2048, n_cores=8, with_bhv=True):
    """Build the SPMD program.  hh = per-core v half width."""
    e = P
    nd2 = dim // (2 * P)   # DoubleRow contraction pair tiles
    njt = seq // P
    njp = njt // 2
    H = 2 * hh             # full hidden width
    nct = H // P           # full-hidden c tiles (gate, og)
    ncp = nct // 2
    nmy = seq // 2         # own query rows per core
    IC = 512
    n_ic = seq // IC
    n_mc = nmy // IC       # own-row chunks
    CC = 512
    n_cc = hh // CC
    DC = 512
    n_dc = dim // DC
    pairs = [[2 * g, 2 * g + 1] for g in range(n_cores // 2)]

    nc = bacc.Bacc("TRN2", target_bir_lowering=False, debug=False,
                   num_devices=n_cores)

    x8g_d = nc.dram_tensor("x8g", [P, nd2, 2, seq], F8, kind="ExternalInput")
    x8my_d = nc.dram_tensor("x8my", [P, nd2, 2, nmy], F8,
                            kind="ExternalInput")
    whv8_d = nc.dram_tensor("whv8", [P, nd2, 2, hh], F8, kind="ExternalInput")
    whg8_d = nc.dram_tensor("whg8", [P, nd2, 2, H], F8, kind="ExternalInput")
    wqk8_d = nc.dram_tensor("wqk8", [P, nd2, 2, e], F8, kind="ExternalInput")
    wout8_d = nc.dram_tensor("wout8", [n_dc, P, ncp, 2, DC], F8,
                             kind="ExternalInput")
    bqk_d = nc.dram_tensor("bqk", [e, 1], F32, kind="ExternalInput")
    bqks_d = nc.dram_tensor("bqks", [e, 1], F32, kind="ExternalInput")
    gq_d = nc.dram_tensor("gq", [e, 1], F32, kind="ExternalInput")
    bq_d = nc.dram_tensor("bq", [e, 1], F32, kind="ExternalInput")
    gk_d = nc.dram_tensor("gk", [e, 1], F32, kind="ExternalInput")
    bk_d = nc.dram_tensor("bk", [e, 1], F32, kind="ExternalInput")
    bhv_d = nc.dram_tensor("bhv", [1, hh], F32, kind="ExternalInput")
    bhgT_d = nc.dram_tensor("bhgT", [P, nct], F32, kind="ExternalInput")
    bhgTs_d = nc.dram_tensor("bhgTs", [P, nct], F32, kind="ExternalInput")
    out_d = nc.dram_tensor("out", [nmy, dim], BF16, kind="ExternalOutput")

    with tile.TileContext(nc) as tc, ExitStack() as st:
        constp = st.enter_context(tc.tile_pool(name="const", bufs=1))
        psump = st.enter_context(tc.tile_pool(name="psum", bufs=8, space="PSUM"))
        dramp = st.enter_context(tc.tile_pool(name="dram", bufs=1, space="DRAM"))
        mainp = st.enter_context(tc.tile_pool(name="main", bufs=1))

        # v AllGather bounce buffers, one per 512-col slab of the own half
        vgi = [dramp.tile([P, njt, CC], F8, tag=f"vgi{k}", name=f"vgi{k}")
               for k in range(n_cc)]
        vgo = [dramp.tile([2, P, njt, CC], F8, tag=f"vgo{k}", name=f"vgo{k}")
               for k in range(n_cc)]

        # ---- constants ----
        wqk8_sb = constp.tile([P, nd2, 2, e], F8, tag="wqk8")
        nc.sync.dma_start(wqk8_sb[:], wqk8_d[:])
        bqk_sb = constp.tile([e, 1], F32, tag="bqk")
        nc.sync.dma_start(bqk_sb[:], bqk_d[:])
        gq_sb = constp.tile([e, 1], F32, tag="gq")
        nc.sync.dma_start(gq_sb[:], gq_d[:])
        bq_sb = constp.tile([e, 1], F32, tag="bq")
        nc.sync.dma_start(bq_sb[:], bq_d[:])
        gk_sb = constp.tile([e, 1], F32, tag="gk")
        nc.sync.dma_start(gk_sb[:], gk_d[:])
        bk_sb = constp.tile([e, 1], F32, tag="bk")
        nc.sync.dma_start(bk_sb[:], bk_d[:])
        bhg_sb = constp.tile([P, nct], F32, tag="bhg")
        nc.sync.dma_start(bhg_sb[:], bhgT_d[:])
        bhv_sb = constp.tile([1, hh], F32, tag="bhv")
        nc.sync.dma_start(bhv_sb[:], bhv_d[:])
        ones_sb = constp.tile([1, P], F32, tag="ones")
        nc.vector.memset(ones_sb[:], 1.0)

        # tiny AllGather to warm the collective stream (first real op
        # otherwise pays a ~50us cold-start)
        warm_in = dramp.tile([1, 64], F32, tag="warm_in", name="warm_in")
        warm_out = dramp.tile([2, 64], F32, tag="warm_out", name="warm_out")
        warm_sb = constp.tile([1, 64], F32, tag="warm_sb")
        nc.vector.memset(warm_sb[:], 0.0)
        nc.gpsimd.dma_start(warm_in[:], warm_sb[:])
        nc.gpsimd.collective_compute("AllGather", ALU.bypass,
                                     replica_groups=pairs,
                                     ins=[warm_in.opt()],
                                     outs=[warm_out.opt()])

        # persistent activations
        qT_sb = mainp.tile([e, nmy], BF16, tag="qT", name="qT")
        kT_sb = mainp.tile([e, seq], BF16, tag="kT", name="kT")
        v8_sb = mainp.tile([P, njt, H], F8, tag="v8", name="v8")
        gate8_sb = mainp.tile([P, nct, nmy], F8, tag="gate8", name="gate8")
        at8_sb = mainp.tile([P, njt, IC], F8, tag="at8", name="at8")

        def scores(c, pool):
            # attnT[j, own chunk c] = relu(sim*sqrt(sa)/seq)^2 -> fp8
            chunk = slice(c * IC, (c + 1) * IC)
            for jt in range(njt):
                ps = psump.tile([P, IC], F32, tag="ps", name="ps")
                nc.tensor.matmul(ps[:], kT_sb[:, jt * P:(jt + 1) * P],
                                 qT_sb[:, chunk], start=True, stop=True)
                rstage = pool.tile([P, IC], BF16, tag="rstage", bufs=4,
                                   name="rstage")
                nc.scalar.activation(rstage[:], ps[:], AF.Relu, scale=RSC)
                nc.vector.tensor_tensor(at8_sb[:, jt, :], rstage[:],
                                        rstage[:], ALU.mult)

        with tc.tile_pool(name="ph1", bufs=1) as ph1p, \
             tc.tile_pool(name="wstream", bufs=1) as wsp:
            x8m_sb = ph1p.tile([P, nd2, 2, nmy], F8, tag="x8m", name="x8m")
            nc.sync.dma_start(x8m_sb[:], x8my_d[:])
            x8g_sb = ph1p.tile([P, nd2, 2, seq], F8, tag="x8g", name="x8g")
            for quar in range(4):
                cols = slice(quar * (seq // 4), (quar + 1) * (seq // 4))
                nc.sync.dma_start(x8g_sb[:, :, :, cols], x8g_d[:, :, :, cols])

            # ---- qk projection (fp8 DoubleRow) ----
            # kT from the global-order x, qT from the own-rows x.
            # silu(u) = u * sigmoid(u); no Silu LUT in the interp, so build
            # from Sigmoid (ACT) + mult (DVE).
            with tc.tile_pool(name="qkp", bufs=1) as qkp:
                qkg_sb = qkp.tile([e, seq], BF16, tag="qkg", name="qkg")
                qkm_sb = qkp.tile([e, nmy], BF16, tag="qkm", name="qkm")
                for n_chunks, xsb, qk_sb in ((n_mc, x8m_sb, qkm_sb),
                                             (n_ic, x8g_sb, qkg_sb)):
                    for ic in range(n_chunks):
                        cols = slice(ic * IC, (ic + 1) * IC)
                        ps = psump.tile([e, IC], F32, tag="ps", name="ps")
                        for dp in range(nd2):
                            nc.tensor.matmul(ps[:], wqk8_sb[:, dp],
                                             xsb[:, dp, :, cols],
                                             start=(dp == 0),
                                             stop=(dp == nd2 - 1),
                                             perf_mode=DR)
                        nc.scalar.activation(qk_sb[:, cols], ps[:], AF.Silu,
                                             bias=bqk_sb[:], scale=INV_H)
                    if qk_sb is qkm_sb:
                        nc.vector.tensor_scalar(qT_sb[:], qkm_sb[:], gq_sb[:],
                                                bq_sb[:], ALU.mult, ALU.add)
                nc.vector.tensor_scalar(kT_sb[:], qkg_sb[:], gk_sb[:],
                                        bk_sb[:], ALU.mult, ALU.add)

            # ---- v half: v[j, c], exchanged via pairwise AllGather ----
            # the AG output is indexed by source rank = hidden-half index,
            # so assembling the full v is rank-independent; the per-slab
            # pipeline hides the exchange behind the gate projection below
            for cc in range(n_cc):
                ccs = slice(cc * CC, (cc + 1) * CC)
                wv8 = [wsp.tile([P, 2, CC], F8, tag=f"wv{dp}", bufs=2,
                                name=f"wv{dp}") for dp in range(nd2)]
                for dp in range(nd2):
                    nc.sync.dma_start(wv8[dp][:], whv8_d[:, dp, :, ccs])
                for jt in range(njt):
                    jts = slice(jt * P, (jt + 1) * P)
                    ps = psump.tile([P, CC], F32, tag="ps", name="ps")
                    for dp in range(nd2):
                        nc.tensor.matmul(ps[:], x8g_sb[:, dp, :, jts],
                                         wv8[dp][:], start=(dp == 0),
                                         stop=(not with_bhv and dp == nd2 - 1),
                                         perf_mode=DR)
                    if with_bhv:
                        nc.tensor.matmul(ps[:], ones_sb[:], bhv_sb[:, ccs],
                                         start=False, stop=True)
                    vs = wsp.tile([P, CC], BF16, tag="vs", bufs=2, name="vs")
                    nc.scalar.activation(vs[:], ps[:], AF.Silu, scale=INV_H)
                    vst = wsp.tile([P, CC], F8, tag="vst", bufs=3, name="vst")
                    nc.vector.tensor_scalar(vst[:], vs[:], SV, None, ALU.mult)
                    nc.scalar.dma_start(vgi[cc][:, jt, :], vst[:])
                nc.gpsimd.collective_compute("AllGather", ALU.bypass,
                                             replica_groups=pairs,
                                             ins=[vgi[cc].opt()],
                                             outs=[vgo[cc].opt()])
                for s in range(2):
                    c0 = s * hh + cc * CC
                    nc.gpsimd.dma_start(v8_sb[:, :, c0:c0 + CC], vgo[cc][s])

            # scores for own chunk 0: the relu/square hide under the gate
            # projection's matmuls
            scores(0, wsp)

            # ---- gate: gateT[c, i] over the FULL hidden, own rows only ----
            for ct in range(nct):
                cts = slice(ct * P, (ct + 1) * P)
                wg8 = [wsp.tile([P, 2, P], F8, tag=f"wg{dp}", bufs=1,
                                name=f"wg{dp}") for dp in range(nd2)]
                for dp in range(nd2):
                    nc.sync.dma_start(wg8[dp][:], whg8_d[:, dp, :, cts])
                for ic in range(n_mc):
                    cols = slice(ic * IC, (ic + 1) * IC)
                    ps = psump.tile([P, IC], F32, tag="ps", name="ps")
                    for dp in range(nd2):
                        nc.tensor.matmul(ps[:], wg8[dp][:],
                                         x8m_sb[:, dp, :, cols],
                                         start=(dp == 0), stop=(dp == nd2 - 1),
                                         perf_mode=DR)
                    gs = wsp.tile([P, IC], BF16, tag="gs", bufs=2, name="gs")
                    nc.scalar.activation(gs[:], ps[:], AF.Silu,
                                         bias=bhg_sb[:, ct:ct + 1],
                                         scale=INV_H)
                    nc.vector.tensor_scalar(gate8_sb[:, ct, cols], gs[:],
                                            KGF, None, ALU.mult)

        # ---- attention + output over own rows ----
        with tc.tile_pool(name="ph2", bufs=1) as ph2p:
            og8_sb = ph2p.tile([P, nct, nmy], F8, tag="og8", name="og8")
            # prefetch the first two W_out slabs during the og matmuls
            wo8 = [ph2p.tile([P, ncp, 2, DC], F8, tag=f"wo8{i}",
                             name=f"wo8{i}") for i in range(2)]
            nc.sync.dma_start(wo8[0][:], wout8_d[0])
            nc.sync.dma_start(wo8[1][:], wout8_d[1])

            def og(c):
                # ogT[c over FULL hidden, own chunk] = (v^T attnT) * gate
                chunk = slice(c * IC, (c + 1) * IC)
                for ct in range(nct):
                    cts = slice(ct * P, (ct + 1) * P)
                    ps = psump.tile([P, IC], F32, tag="ps", name="ps")
                    for jp in range(njp):
                        nc.tensor.matmul(ps[:],
                                         v8_sb[:, 2 * jp:2 * jp + 2, cts],
                                         at8_sb[:, 2 * jp:2 * jp + 2, :],
                                         start=(jp == 0), stop=(jp == njp - 1),
                                         perf_mode=DR)
                    t2 = ph2p.tile([P, IC], BF16, tag="t2", bufs=2, name="t2")
                    nc.vector.tensor_scalar(t2[:], ps[:], OG_SC, None,
                                            ALU.mult)
                    nc.vector.tensor_tensor(og8_sb[:, ct, chunk], t2[:],
                                            gate8_sb[:, ct, chunk], ALU.mult)

            og(0)
            scores(1, ph2p)   # relu/square hide under og(0)'s matmuls
            og(1)
            # branch[own rows, :] = ogT^T @ Wout (full 4096 contraction),
            # fp8 W_out streamed per 512-col output slab, double-buffered
            # two slabs ahead, straight to the output
            for dc in range(n_dc):
                dcs = slice(dc * DC, (dc + 1) * DC)
                w = wo8[dc % 2]
                for it in range(nmy // P):
                    its = slice(it * P, (it + 1) * P)
                    ps = psump.tile([P, DC], F32, tag="ps", name="ps")
                    for cp in range(ncp):
                        nc.tensor.matmul(ps[:],
                                         og8_sb[:, 2 * cp:2 * cp + 2, its],
                                         w[:, cp, :, :],
                                         start=(cp == 0),
                                         stop=(cp == ncp - 1),
                                         perf_mode=DR)
                    po = ph2p.tile([P, DC], BF16, tag="po", bufs=4, name="po")
                    nc.vector.tensor_scalar(po[:], ps[:], INV_O, None,
                                            ALU.mult)
                    nc.scalar.dma_start(out_d[it * P:(it + 1) * P, dcs],
                                        po[:])
                if dc + 2 < n_dc:
                    nc.sync.dma_start(w[:], wout8_d[dc + 2])

    nc.compile()
    return nc


def own_rows(seq, h):
    """Rows owned by pair-member h: contiguous half of the sequence."""
    nmy = seq // 2
    return np.arange(h * nmy, (h + 1) * nmy)


def _q8(a, s):
    """Quantize a*s to fp8 e4m3, clipping to +-240 (TRN e4m3 infs at 256)."""
    return np.clip(a.astype(np.float32) * s, -F8MAX, F8MAX).astype(
        ml_dtypes.float8_e4m3)


def _dr_pack(m8):
    """[K, F] fp8 -> [128, K//256, 2, F] DoubleRow pair layout."""
    K, F = m8.shape
    return np.ascontiguousarray(
        m8.reshape(K // 256, 2, P, F).transpose(2, 0, 1, 3))


def make_in_maps(x, W_hidden, b_hidden, W_qk, b_qk, gamma_q, beta_q,
                 gamma_k, beta_k, W_out, b_out, n_cores=8):
    """Host-side quantization/layout prep.  Returns per-core input dicts."""
    B, seq, dim = x.shape
    H2 = W_hidden.shape[1]
    H = H2 // 2
    hh = H // 2
    nct = H // P
    nmy = seq // 2
    f32 = np.float32
    x8_cache = {}
    xm_cache = {}
    half_cache = {}
    # full-hidden tensors, shared by both pair members
    whg8 = _dr_pack(_q8(W_hidden[:, H:], SW))
    _wo = _dr_pack(_q8(W_out, SWO))
    wout8 = np.ascontiguousarray(np.stack(
        [_wo[:, :, :, dc * 512:(dc + 1) * 512] for dc in range(dim // 512)]))
    wqk8 = _dr_pack(_q8(W_qk, SW))
    bhgT = np.ascontiguousarray(b_hidden[H:].astype(f32).reshape(nct, P).T)

    def halves(h):
        if h not in half_cache:
            cs = slice(h * hh, (h + 1) * hh)
            half_cache[h] = {
                "whv8": _dr_pack(_q8(W_hidden[:, cs], SW)),
                "bhv": (b_hidden[cs].astype(f32) * (SX * SW)).reshape(1, -1),
            }
        return half_cache[h]

    in_maps = []
    for core in range(n_cores):
        b, h = core // 2, core % 2
        if b not in x8_cache:
            x8_cache[b] = _dr_pack(_q8(np.ascontiguousarray(x[b].T), SX))
        if (b, h) not in xm_cache:
            xm_cache[(b, h)] = np.ascontiguousarray(
                x8_cache[b][:, :, :, h * nmy:(h + 1) * nmy])
        hv = halves(h)
        in_maps.append({
            "x8g": x8_cache[b],
            "x8my": xm_cache[(b, h)],
            "whv8": hv["whv8"],
            "whg8": whg8,
            "wout8": wout8,
            "wqk8": wqk8,
            "bqk": b_qk.reshape(-1, 1).astype(f32),
            "bqks": (b_qk.reshape(-1, 1).astype(f32) * (SX * SW)),
            "gq": gamma_q.reshape(-1, 1).astype(f32),
            "bq": beta_q.reshape(-1, 1).astype(f32),
            "gk": gamma_k.reshape(-1, 1).astype(f32),
            "bk": beta_k.reshape(-1, 1).astype(f32),
            "bhv": hv["bhv"],
            "bhgT": bhgT,
            "bhgTs": bhgT * (SX * SW),
        })
    return in_maps


_NC_CACHE = {}


def _get_nc(seq, dim, hh, n_cores, with_bhv=True):
    key = (seq, dim, hh, n_cores, with_bhv)
    if key not in _NC_CACHE:
        _NC_CACHE[key] = build_gau_nc(seq=seq, dim=dim, hh=hh,
                                      n_cores=n_cores, with_bhv=with_bhv)
    return _NC_CACHE[key]


def kernel(x, W_hidden, b_hidden, W_qk, b_qk, gamma_q, beta_q, gamma_k,
           beta_k, W_out, b_out):
    x = np.asarray(x)
    B, seq, dim = x.shape
    hh = W_hidden.shape[1] // 4
    n_cores = 2 * B
    with_bhv = bool(np.any(np.asarray(b_hidden)[: 2 * hh] != 0))
    nc = _get_nc(seq, dim, hh, n_cores, with_bhv=with_bhv)
    in_maps = make_in_maps(x, np.asarray(W_hidden), np.asarray(b_hidden),
                           np.asarray(W_qk), np.asarray(b_qk),
                           np.asarray(gamma_q), np.asarray(beta_q),
                           np.asarray(gamma_k), np.asarray(beta_k),
                           np.asarray(W_out), np.asarray(b_out),
                           n_cores=n_cores)
    res = run_bass_kernel_spmd(nc, in_maps, core_ids=list(range(n_cores)))
    bo = np.asarray(b_out).astype(np.float32)
    out = np.empty((B, seq, dim), np.float32)
    for b in range(B):
        for h in range(2):
            rows = own_rows(seq, h)
            out[b, rows] = (res.results[2 * b + h]["out"].astype(np.float32)
                            + x[b][rows] + bo)
    return out


# revision 16
# speedup vs baseline: 1.0439x; 1.0082x over previous
"""GAU (Gated Attention Unit, relu^2 attention) Trainium2 Bass kernel, fp8.

Problem shapes: x [4, 2048, 2048] f32; W_hidden [2048, 8192]; W_qk [2048, 128];
W_out [4096, 2048]; out = GAU(x) + x.

Sharding (8 cores): core = 2*batch + h, h in {0,1}.  Each pair of cores
handles one batch.  The VALUE/GATE hidden projections are column-split in
half within the pair (core h computes v for hidden cols
[h*2048:(h+1)*2048]); the computed v half is then exchanged with a
pairwise AllGather (fp8, 4MB, pipelined per 512-column slab behind the
gate projection) so both cores hold the full [2048, 4096] v.  An
AllGather works here precisely because both cores want the SAME full v
-- its output is indexed by source rank, which equals the hidden-half
index, keeping the program rank-independent.

Everything downstream is split by QUERY rows instead: core h owns the
global rows [h*1024, (h+1)*1024) and computes scores, attn, gate, og and
the output projection (full 4096-deep contraction, W_out streamed fp8)
only for those rows, writing final bf16 branch rows straight to its
output.  The row split is baked in on the host: a second input tensor
x8my carries only the core's own 1024 x columns for the q projection and
the gate's moving operand, while x8g (global order) feeds k, and v.
There is no partial-sum ReduceScatter and no serial collective tail.
The host adds the residual x and b_out in fp32 (the branch is ~0.3% of
the output norm, so a bf16 branch costs ~3e-6 relative error).

All projections run as fp8e4 (e4m3, max +-240) DoubleRow matmuls: both
operands are packed [128, 2, free] pairing two 128-row contraction
tiles, giving 2 fp8 MACs/PE-cell/cycle.  Quantization scales are powers
of two, chosen so every fp8 tensor's max is >=2.7x below 240 (measured
on the real data distribution; host clips before casting to avoid
e4m3's non-saturating inf at 256):

  x*16, W_hidden*512, W_qk*512, W_out*512, v*16, attn*2^14, gate*32,
  og*1024

PSUM stays fp32; descales fold into the activation `scale` ports and DVE
tensor_scalar immediates.  silu runs on the scalar engine's native Silu
LUT (one ACT op; the CoreSim interp lacks the LUT, so this kernel is
hardware-only).  qT/kT and the relu^2 scores matmul stay bf16 (K=128,
cheap; scores are NOT pair-replicated since each core only needs its own
rows).  The scores for chunk 0 are emitted between v and gate, and for
chunk 1 between og(0) and og(1), hiding their scalar-engine relu under
matmul-heavy phases.  W_out streams per 512-column output slab from a
slab-contiguous DRAM layout, double-buffered two slabs ahead.
"""

import numpy as np
import ml_dtypes
from contextlib import ExitStack

import concourse.bass as bass
import concourse.bacc as bacc
import concourse.mybir as mybir
import concourse.tile as tile
from concourse.bass_utils import run_bass_kernel_spmd

BF16 = mybir.dt.bfloat16
F32 = mybir.dt.float32
F8 = mybir.dt.float8e4
AF = mybir.ActivationFunctionType
ALU = mybir.AluOpType
DR = mybir.MatmulPerfMode.DoubleRow
P = 128
F8MAX = 240.0

# quantization scales (powers of two; see module docstring)
SX = 16.0       # x
SW = 512.0      # W_hidden (both halves), W_qk
SWO = 512.0     # W_out
SV = 16.0       # v activations
SA = 16384.0    # attn = relu(sim)^2    (sqrt(SA) = 128 folds into the relu)
KGF = 32.0      # gate activations
SO = 1024.0     # og activations

INV_H = 1.0 / (SX * SW)        # hidden/qk psum -> real pre-activation
T_V = SV / (SX * SW)           # v-path u multiplier
T_G = KGF / (SX * SW)          # gate-path u multiplier
OG_SC = SO / (SA * SV * KGF)   # og = (psum_av * OG_SC) * gate8
RSC = 128.0 / 2048.0           # sqrt(SA)/seq relu scale
INV_O = 1.0 / (SO * SWO)       # out psum -> real branch rows


def build_gau_nc(seq=2048, dim=2048, hh=2048, n_cores=8, with_bhv=True):
    """Build the SPMD program.  hh = per-core v half width."""
    e = P
    nd2 = dim // (2 * P)   # DoubleRow contraction pair tiles
    njt = seq // P
    njp = njt // 2
    H = 2 * hh             # full hidden width
    nct = H // P           # full-hidden c tiles (gate, og)
    ncp = nct // 2
    nmy = seq // 2         # own query rows per core
    IC = 512
    n_ic = seq // IC
    n_mc = nmy // IC       # own-row chunks
    CC = 512
    n_cc = hh // CC
    DC = 512
    n_dc = dim // DC
    pairs = [[2 * g, 2 * g + 1] for g in range(n_cores // 2)]

    nc = bacc.Bacc("TRN2", target_bir_lowering=False, debug=False,
                   num_devices=n_cores)

    x8g_d = nc.dram_tensor("x8g", [P, nd2, 2, seq], F8, kind="ExternalInput")
    x8my_d = nc.dram_tensor("x8my", [P, nd2, 2, nmy], F8,
                            kind="ExternalInput")
    whv8_d = nc.dram_tensor("whv8", [P, nd2, 2, hh], F8, kind="ExternalInput")
    whg8_d = nc.dram_tensor("whg8", [P, nd2, 2, H], F8, kind="ExternalInput")
    wqk8_d = nc.dram_tensor("wqk8", [P, nd2, 2, e], F8, kind="ExternalInput")
    wout8_d = nc.dram_tensor("wout8", [n_dc, P, ncp, 2, DC], F8,
                             kind="ExternalInput")
    bqk_d = nc.dram_tensor("bqk", [e, 1], F32, kind="ExternalInput")
    bqks_d = nc.dram_tensor("bqks", [e, 1], F32, kind="ExternalInput")
    gq_d = nc.dram_tensor("gq", [e, 1], F32, kind="ExternalInput")
    bq_d = nc.dram_tensor("bq", [e, 1], F32, kind="ExternalInput")
    gk_d = nc.dram_tensor("gk", [e, 1], F32, kind="ExternalInput")
    bk_d = nc.dram_tensor("bk", [e, 1], F32, kind="ExternalInput")
    bhv_d = nc.dram_tensor("bhv", [1, hh], F32, kind="ExternalInput")
    bhgT_d = nc.dram_tensor("bhgT", [P, nct], F32, kind="ExternalInput")
    bhgTs_d = nc.dram_tensor("bhgTs", [P, nct], F32, kind="ExternalInput")
    out_d = nc.dram_tensor("out", [nmy, dim], BF16, kind="ExternalOutput")

    with tile.TileContext(nc) as tc, ExitStack() as st:
        constp = st.enter_context(tc.tile_pool(name="const", bufs=1))
        psump = st.enter_context(tc.tile_pool(name="psum", bufs=8, space="PSUM"))
        dramp = st.enter_context(tc.tile_pool(name="dram", bufs=1, space="DRAM"))
        mainp = st.enter_context(tc.tile_pool(name="main", bufs=1))

        # v AllGather bounce buffers, one per 512-col slab of the own half
        vgi = [dramp.tile([P, njt, CC], F8, tag=f"vgi{k}", name=f"vgi{k}")
               for k in range(n_cc)]
        vgo = [dramp.tile([2, P, njt, CC], F8, tag=f"vgo{k}", name=f"vgo{k}")
               for k in range(n_cc)]

        # ---- constants ----
        wqk8_sb = constp.tile([P, nd2, 2, e], F8, tag="wqk8")
        nc.sync.dma_start(wqk8_sb[:], wqk8_d[:])
        bqk_sb = constp.tile([e, 1], F32, tag="bqk")
        nc.sync.dma_start(bqk_sb[:], bqk_d[:])
        gq_sb = constp.tile([e, 1], F32, tag="gq")
        nc.sync.dma_start(gq_sb[:], gq_d[:])
        bq_sb = constp.tile([e, 1], F32, tag="bq")
        nc.sync.dma_start(bq_sb[:], bq_d[:])
        gk_sb = constp.tile([e, 1], F32, tag="gk")
        nc.sync.dma_start(gk_sb[:], gk_d[:])
        bk_sb = constp.tile([e, 1], F32, tag="bk")
        nc.sync.dma_start(bk_sb[:], bk_d[:])
        bhg_sb = constp.tile([P, nct], F32, tag="bhg")
        nc.sync.dma_start(bhg_sb[:], bhgT_d[:])
        bhv_sb = constp.tile([1, hh], F32, tag="bhv")
        nc.sync.dma_start(bhv_sb[:], bhv_d[:])
        ones_sb = constp.tile([1, P], F32, tag="ones")
        nc.vector.memset(ones_sb[:], 1.0)

        # tiny AllGather to warm the collective stream (first real op
        # otherwise pays a ~50us cold-start)
        warm_in = dramp.tile([1, 64], F32, tag="warm_in", name="warm_in")
        warm_out = dramp.tile([2, 64], F32, tag="warm_out", name="warm_out")
        warm_sb = constp.tile([1, 64], F32, tag="warm_sb")
        nc.vector.memset(warm_sb[:], 0.0)
        nc.gpsimd.dma_start(warm_in[:], warm_sb[:])
        nc.gpsimd.collective_compute("AllGather", ALU.bypass,
                                     replica_groups=pairs,
                                     ins=[warm_in.opt()],
                                     outs=[warm_out.opt()])

        # persistent activations
        qT_sb = mainp.tile([e, nmy], BF16, tag="qT", name="qT")
        kT_sb = mainp.tile([e, seq], BF16, tag="kT", name="kT")
        v8_sb = mainp.tile([P, njt, H], F8, tag="v8", name="v8")
        gate8_sb = mainp.tile([P, nct, nmy], F8, tag="gate8", name="gate8")
        at8_sb = mainp.tile([P, njt, IC], F8, tag="at8", name="at8")

        def scores(c, pool):
            # attnT[j, own chunk c] = relu(sim*sqrt(sa)/seq)^2 -> fp8
            chunk = slice(c * IC, (c + 1) * IC)
            for jt in range(njt):
                ps = psump.tile([P, IC], F32, tag="ps", name="ps")
                nc.tensor.matmul(ps[:], kT_sb[:, jt * P:(jt + 1) * P],
                                 qT_sb[:, chunk], start=True, stop=True)
                rstage = pool.tile([P, IC], BF16, tag="rstage", bufs=4,
                                   name="rstage")
                nc.scalar.activation(rstage[:], ps[:], AF.Relu, scale=RSC)
                nc.vector.tensor_tensor(at8_sb[:, jt, :], rstage[:],
                                        rstage[:], ALU.mult)

        with tc.tile_pool(name="ph1", bufs=1) as ph1p, \
             tc.tile_pool(name="wstream", bufs=1) as wsp, \
             tc.tile_pool(name="qkp", bufs=1) as qkp:
            x8m_sb = ph1p.tile([P, nd2, 2, nmy], F8, tag="x8m", name="x8m")
            nc.sync.dma_start(x8m_sb[:], x8my_d[:])
            x8g_sb = ph1p.tile([P, nd2, 2, seq], F8, tag="x8g", name="x8g")
            for quar in range(4):
                cols = slice(quar * (seq // 4), (quar + 1) * (seq // 4))
                nc.sync.dma_start(x8g_sb[:, :, :, cols], x8g_d[:, :, :, cols])

            # ---- qk projection (fp8 DoubleRow) ----
            # kT from the global-order x, qT from the own-rows x.
            # silu(u) = u * sigmoid(u); no Silu LUT in the interp, so build
            # from Sigmoid (ACT) + mult (DVE).
            qkg_sb = qkp.tile([e, seq], BF16, tag="qkg", name="qkg")
            qkm_sb = qkp.tile([e, nmy], BF16, tag="qkm", name="qkm")

            def qk_proj(n_chunks, xsb, qk_sb):
                for ic in range(n_chunks):
                    cols = slice(ic * IC, (ic + 1) * IC)
                    ps = psump.tile([e, IC], F32, tag="ps", name="ps")
                    for dp in range(nd2):
                        nc.tensor.matmul(ps[:], wqk8_sb[:, dp],
                                         xsb[:, dp, :, cols],
                                         start=(dp == 0),
                                         stop=(dp == nd2 - 1),
                                         perf_mode=DR)
                    nc.scalar.activation(qk_sb[:, cols], ps[:], AF.Silu,
                                         bias=bqk_sb[:], scale=INV_H)

            qk_proj(n_ic, x8g_sb, qkg_sb)
            nc.vector.tensor_scalar(kT_sb[:], qkg_sb[:], gk_sb[:],
                                    bk_sb[:], ALU.mult, ALU.add)

            # ---- v half: v[j, c], exchanged via pairwise AllGather ----
            # the AG output is indexed by source rank = hidden-half index,
            # so assembling the full v is rank-independent; the per-slab
            # pipeline hides the exchange behind the gate projection below
            for cc in range(n_cc):
                ccs = slice(cc * CC, (cc + 1) * CC)
                wv8 = [wsp.tile([P, 2, CC], F8, tag=f"wv{dp}", bufs=2,
                                name=f"wv{dp}") for dp in range(nd2)]
                for dp in range(nd2):
                    nc.sync.dma_start(wv8[dp][:], whv8_d[:, dp, :, ccs])
                for jt in range(njt):
                    jts = slice(jt * P, (jt + 1) * P)
                    ps = psump.tile([P, CC], F32, tag="ps", name="ps")
                    for dp in range(nd2):
                        nc.tensor.matmul(ps[:], x8g_sb[:, dp, :, jts],
                                         wv8[dp][:], start=(dp == 0),
                                         stop=(not with_bhv and dp == nd2 - 1),
                                         perf_mode=DR)
                    if with_bhv:
                        nc.tensor.matmul(ps[:], ones_sb[:], bhv_sb[:, ccs],
                                         start=False, stop=True)
                    vs = wsp.tile([P, CC], BF16, tag="vs", bufs=2, name="vs")
                    nc.scalar.activation(vs[:], ps[:], AF.Silu, scale=INV_H)
                    vst = wsp.tile([P, CC], F8, tag="vst", bufs=3, name="vst")
                    nc.vector.tensor_scalar(vst[:], vs[:], SV, None, ALU.mult)
                    nc.scalar.dma_start(vgi[cc][:, jt, :], vst[:])
                nc.gpsimd.collective_compute("AllGather", ALU.bypass,
                                             replica_groups=pairs,
                                             ins=[vgi[cc].opt()],
                                             outs=[vgo[cc].opt()])
                for s in range(2):
                    c0 = s * hh + cc * CC
                    nc.gpsimd.dma_start(v8_sb[:, :, c0:c0 + CC], vgo[cc][s])

            qk_proj(n_mc, x8m_sb, qkm_sb)
            nc.vector.tensor_scalar(qT_sb[:], qkm_sb[:], gq_sb[:],
                                    bq_sb[:], ALU.mult, ALU.add)
            # scores for own chunk 0: the relu/square hide under the gate
            # projection's matmuls
            scores(0, wsp)

            # ---- gate: gateT[c, i] over the FULL hidden, own rows only ----
            for ct in range(nct):
                cts = slice(ct * P, (ct + 1) * P)
                wg8 = [wsp.tile([P, 2, P], F8, tag=f"wg{dp}", bufs=2,
                                name=f"wg{dp}") for dp in range(nd2)]
                for dp in range(nd2):
                    nc.sync.dma_start(wg8[dp][:], whg8_d[:, dp, :, cts])
                for ic in range(n_mc):
                    cols = slice(ic * IC, (ic + 1) * IC)
                    ps = psump.tile([P, IC], F32, tag="ps", name="ps")
                    for dp in range(nd2):
                        nc.tensor.matmul(ps[:], wg8[dp][:],
                                         x8m_sb[:, dp, :, cols],
                                         start=(dp == 0), stop=(dp == nd2 - 1),
                                         perf_mode=DR)
                    gs = wsp.tile([P, IC], BF16, tag="gs", bufs=2, name="gs")
                    nc.scalar.activation(gs[:], ps[:], AF.Silu,
                                         bias=bhg_sb[:, ct:ct + 1],
                                         scale=INV_H)
                    nc.vector.tensor_scalar(gate8_sb[:, ct, cols], gs[:],
                                            KGF, None, ALU.mult)

        # ---- attention + output over own rows ----
        with tc.tile_pool(name="ph2", bufs=1) as ph2p:
            og8_sb = ph2p.tile([P, nct, nmy], F8, tag="og8", name="og8")
            # prefetch the first two W_out slabs during the og matmuls
            wo8 = [ph2p.tile([P, ncp, 2, DC], F8, tag=f"wo8{i}",
                             name=f"wo8{i}") for i in range(2)]
            nc.sync.dma_start(wo8[0][:], wout8_d[0])
            nc.sync.dma_start(wo8[1][:], wout8_d[1])

            def og(c):
                # ogT[c over FULL hidden, own chunk] = (v^T attnT) * gate
                chunk = slice(c * IC, (c + 1) * IC)
                for ct in range(nct):
                    cts = slice(ct * P, (ct + 1) * P)
                    ps = psump.tile([P, IC], F32, tag="ps", name="ps")
                    for jp in range(njp):
                        nc.tensor.matmul(ps[:],
                                         v8_sb[:, 2 * jp:2 * jp + 2, cts],
                                         at8_sb[:, 2 * jp:2 * jp + 2, :],
                                         start=(jp == 0), stop=(jp == njp - 1),
                                         perf_mode=DR)
                    t2 = ph2p.tile([P, IC], BF16, tag="t2", bufs=2, name="t2")
                    nc.vector.tensor_scalar(t2[:], ps[:], OG_SC, None,
                                            ALU.mult)
                    nc.vector.tensor_tensor(og8_sb[:, ct, chunk], t2[:],
                                            gate8_sb[:, ct, chunk], ALU.mult)

            og(0)
            scores(1, ph2p)   # relu/square hide under og(0)'s matmuls
            og(1)
            # branch[own rows, :] = ogT^T @ Wout (full 4096 contraction),
            # fp8 W_out streamed per 512-col output slab, double-buffered
            # two slabs ahead, straight to the output
            for dc in range(n_dc):
                dcs = slice(dc * DC, (dc + 1) * DC)
                w = wo8[dc % 2]
                for it in range(nmy // P):
                    its = slice(it * P, (it + 1) * P)
                    ps = psump.tile([P, DC], F32, tag="ps", name="ps")
                    for cp in range(ncp):
                        nc.tensor.matmul(ps[:],
                                         og8_sb[:, 2 * cp:2 * cp + 2, its],
                                         w[:, cp, :, :],
                                         start=(cp == 0),
                                         stop=(cp == ncp - 1),
                                         perf_mode=DR)
                    po = ph2p.tile([P, DC], BF16, tag="po", bufs=4, name="po")
                    nc.vector.tensor_scalar(po[:], ps[:], INV_O, None,
                                            ALU.mult)
                    nc.scalar.dma_start(out_d[it * P:(it + 1) * P, dcs],
                                        po[:])
                if dc + 2 < n_dc:
                    nc.sync.dma_start(w[:], wout8_d[dc + 2])

    nc.compile()
    return nc


def own_rows(seq, h):
    """Rows owned by pair-member h: contiguous half of the sequence."""
    nmy = seq // 2
    return np.arange(h * nmy, (h + 1) * nmy)


def _q8(a, s):
    """Quantize a*s to fp8 e4m3, clipping to +-240 (TRN e4m3 infs at 256)."""
    return np.clip(a.astype(np.float32) * s, -F8MAX, F8MAX).astype(
        ml_dtypes.float8_e4m3)


def _dr_pack(m8):
    """[K, F] fp8 -> [128, K//256, 2, F] DoubleRow pair layout."""
    K, F = m8.shape
    return np.ascontiguousarray(
        m8.reshape(K // 256, 2, P, F).transpose(2, 0, 1, 3))


def make_in_maps(x, W_hidden, b_hidden, W_qk, b_qk, gamma_q, beta_q,
                 gamma_k, beta_k, W_out, b_out, n_cores=8):
    """Host-side quantization/layout prep.  Returns per-core input dicts."""
    B, seq, dim = x.shape
    H2 = W_hidden.shape[1]
    H = H2 // 2
    hh = H // 2
    nct = H // P
    nmy = seq // 2
    f32 = np.float32
    x8_cache = {}
    xm_cache = {}
    half_cache = {}
    # full-hidden tensors, shared by both pair members
    whg8 = _dr_pack(_q8(W_hidden[:, H:], SW))
    _wo = _dr_pack(_q8(W_out, SWO))
    wout8 = np.ascontiguousarray(np.stack(
        [_wo[:, :, :, dc * 512:(dc + 1) * 512] for dc in range(dim // 512)]))
    wqk8 = _dr_pack(_q8(W_qk, SW))
    bhgT = np.ascontiguousarray(b_hidden[H:].astype(f32).reshape(nct, P).T)

    def halves(h):
        if h not in half_cache:
            cs = slice(h * hh, (h + 1) * hh)
            half_cache[h] = {
                "whv8": _dr_pack(_q8(W_hidden[:, cs], SW)),
                "bhv": (b_hidden[cs].astype(f32) * (SX * SW)).reshape(1, -1),
            }
        return half_cache[h]

    in_maps = []
    for core in range(n_cores):
        b, h = core // 2, core % 2
        if b not in x8_cache:
            x8_cache[b] = _dr_pack(_q8(np.ascontiguousarray(x[b].T), SX))
        if (b, h) not in xm_cache:
            xm_cache[(b, h)] = np.ascontiguousarray(
                x8_cache[b][:, :, :, h * nmy:(h + 1) * nmy])
        hv = halves(h)
        in_maps.append({
            "x8g": x8_cache[b],
            "x8my": xm_cache[(b, h)],
            "whv8": hv["whv8"],
            "whg8": whg8,
            "wout8": wout8,
            "wqk8": wqk8,
            "bqk": b_qk.reshape(-1, 1).astype(f32),
            "bqks": (b_qk.reshape(-1, 1).astype(f32) * (SX * SW)),
            "gq": gamma_q.reshape(-1, 1).astype(f32),
            "bq": beta_q.reshape(-1, 1).astype(f32),
            "gk": gamma_k.reshape(-1, 1).astype(f32),
            "bk": beta_k.reshape(-1, 1).astype(f32),
            "bhv": hv["bhv"],
            "bhgT": bhgT,
            "bhgTs": bhgT * (SX * SW),
        })
    return in_maps


_NC_CACHE = {}


def _get_nc(seq, dim, hh, n_cores, with_bhv=True):
    key = (seq, dim, hh, n_cores, with_bhv)
    if key not in _NC_CACHE:
        _NC_CACHE[key] = build_gau_nc(seq=seq, dim=dim, hh=hh,
                                      n_cores=n_cores, with_bhv=with_bhv)
    return _NC_CACHE[key]


def kernel(x, W_hidden, b_hidden, W_qk, b_qk, gamma_q, beta_q, gamma_k,
           beta_k, W_out, b_out):
    x = np.asarray(x)
    B, seq, dim = x.shape
    hh = W_hidden.shape[1] // 4
    n_cores = 2 * B
    with_bhv = bool(np.any(np.asarray(b_hidden)[: 2 * hh] != 0))
    nc = _get_nc(seq, dim, hh, n_cores, with_bhv=with_bhv)
    in_maps = make_in_maps(x, np.asarray(W_hidden), np.asarray(b_hidden),
                           np.asarray(W_qk), np.asarray(b_qk),
                           np.asarray(gamma_q), np.asarray(beta_q),
                           np.asarray(gamma_k), np.asarray(beta_k),
                           np.asarray(W_out), np.asarray(b_out),
                           n_cores=n_cores)
    res = run_bass_kernel_spmd(nc, in_maps, core_ids=list(range(n_cores)))
    bo = np.asarray(b_out).astype(np.float32)
    out = np.empty((B, seq, dim), np.float32)
    for b in range(B):
        for h in range(2):
            rows = own_rows(seq, h)
            out[b, rows] = (res.results[2 * b + h]["out"].astype(np.float32)
                            + x[b][rows] + bo)
    return out


# revision 17
# speedup vs baseline: 1.1336x; 1.0860x over previous
"""GAU (Gated Attention Unit, relu^2 attention) Trainium2 Bass kernel, fp8.

Problem shapes: x [4, 2048, 2048] f32; W_hidden [2048, 8192]; W_qk [2048, 128];
W_out [4096, 2048]; out = GAU(x) + x.

Sharding (8 cores): core = 2*batch + h, h in {0,1}.  Each pair of cores
handles one batch.  The VALUE/GATE hidden projections are column-split in
half within the pair (core h computes v for hidden cols
[h*2048:(h+1)*2048]); the computed v half is then exchanged with a
pairwise AllGather (fp8, 4MB, pipelined per 512-column slab behind the
gate projection) so both cores hold the full [2048, 4096] v.  An
AllGather works here precisely because both cores want the SAME full v
-- its output is indexed by source rank, which equals the hidden-half
index, keeping the program rank-independent.

Everything downstream is split by QUERY rows instead: core h owns the
global rows [h*1024, (h+1)*1024) and computes scores, attn, gate, og and
the output projection (full 4096-deep contraction, W_out streamed fp8)
only for those rows, writing final bf16 branch rows straight to its
output.  The row split is baked in on the host: a second input tensor
x8my carries only the core's own 1024 x columns for the q projection and
the gate's moving operand, while x8g (global order) feeds k, and v.
There is no partial-sum ReduceScatter and no serial collective tail.
The host adds the residual x and b_out in fp32 (the branch is ~0.3% of
the output norm, so a bf16 branch costs ~3e-6 relative error).

All projections run as fp8e4 (e4m3, max +-240) DoubleRow matmuls: both
operands are packed [128, 2, free] pairing two 128-row contraction
tiles, giving 2 fp8 MACs/PE-cell/cycle.  Quantization scales are powers
of two, chosen so every fp8 tensor's max is >=2.7x below 240 (measured
on the real data distribution; host clips before casting to avoid
e4m3's non-saturating inf at 256):

  x*16, W_hidden*512, W_qk*512, W_out*512, v*16, attn*2^14, gate*32,
  og*1024

PSUM stays fp32; descales fold into the activation `scale` ports and DVE
tensor_scalar immediates.  silu runs on the scalar engine's native Silu
LUT (one ACT op; the CoreSim interp lacks the LUT, so this kernel is
hardware-only).  qT/kT and the relu^2 scores matmul stay bf16 (K=128,
cheap; scores are NOT pair-replicated since each core only needs its own
rows).  The scores for chunk 0 are emitted between v and gate, and for
chunk 1 between og(0) and og(1), hiding their scalar-engine relu under
matmul-heavy phases.  W_out streams per 512-column output slab from a
slab-contiguous DRAM layout, double-buffered two slabs ahead.
"""

import numpy as np
import ml_dtypes
from contextlib import ExitStack

import concourse.bass as bass
import concourse.bacc as bacc
import concourse.mybir as mybir
import concourse.tile as tile
from concourse.bass_utils import run_bass_kernel_spmd

BF16 = mybir.dt.bfloat16
F32 = mybir.dt.float32
F8 = mybir.dt.float8e4
AF = mybir.ActivationFunctionType
ALU = mybir.AluOpType
DR = mybir.MatmulPerfMode.DoubleRow
P = 128
F8MAX = 240.0

# quantization scales (powers of two; see module docstring)
SX = 16.0       # x
SW = 512.0      # W_hidden (both halves), W_qk
SWO = 512.0     # W_out
SV = 16.0       # v activations
SA = 16384.0    # attn = relu(sim)^2    (sqrt(SA) = 128 folds into the relu)
KGF = 32.0      # gate activations
SO = 1024.0     # og activations

INV_H = 1.0 / (SX * SW)        # hidden/qk psum -> real pre-activation
T_V = SV / (SX * SW)           # v-path u multiplier
T_G = KGF / (SX * SW)          # gate-path u multiplier
OG_SC = SO / (SA * SV * KGF)   # og = (psum_av * OG_SC) * gate8
RSC = 128.0 / 2048.0           # sqrt(SA)/seq relu scale
INV_O = 1.0 / (SO * SWO)       # out psum -> real branch rows


def build_gau_nc(seq=2048, dim=2048, hh=2048, n_cores=8, with_bhv=True):
    """Build the SPMD program.  hh = per-core v half width."""
    e = P
    nd2 = dim // (2 * P)   # DoubleRow contraction pair tiles
    njt = seq // P
    njp = njt // 2
    H = 2 * hh             # full hidden width
    nct = H // P           # full-hidden c tiles (gate, og)
    ncp = nct // 2
    nmy = seq // 2         # own query rows per core
    IC = 512
    n_ic = seq // IC
    n_mc = nmy // IC       # own-row chunks
    CC = 512
    n_cc = hh // CC
    DC = 512
    n_dc = dim // DC
    pairs = [[2 * g, 2 * g + 1] for g in range(n_cores // 2)]

    nc = bacc.Bacc("TRN2", target_bir_lowering=False, debug=False,
                   num_devices=n_cores)

    x8g_d = nc.dram_tensor("x8g", [P, nd2, 2, seq], F8, kind="ExternalInput")
    x8my_d = nc.dram_tensor("x8my", [P, nd2, 2, nmy], F8,
                            kind="ExternalInput")
    whv8_d = nc.dram_tensor("whv8", [n_cc, P, nd2, 2, CC], F8,
                            kind="ExternalInput")
    whg8_d = nc.dram_tensor("whg8", [nct, P, nd2, 2, P], F8,
                            kind="ExternalInput")
    wqk8_d = nc.dram_tensor("wqk8", [P, nd2, 2, e], F8, kind="ExternalInput")
    wout8_d = nc.dram_tensor("wout8", [n_dc, P, ncp, 2, DC], F8,
                             kind="ExternalInput")
    bqk_d = nc.dram_tensor("bqk", [e, 1], F32, kind="ExternalInput")
    bqks_d = nc.dram_tensor("bqks", [e, 1], F32, kind="ExternalInput")
    gq_d = nc.dram_tensor("gq", [e, 1], F32, kind="ExternalInput")
    bq_d = nc.dram_tensor("bq", [e, 1], F32, kind="ExternalInput")
    gk_d = nc.dram_tensor("gk", [e, 1], F32, kind="ExternalInput")
    bk_d = nc.dram_tensor("bk", [e, 1], F32, kind="ExternalInput")
    bhv_d = nc.dram_tensor("bhv", [1, hh], F32, kind="ExternalInput")
    bhgT_d = nc.dram_tensor("bhgT", [P, nct], F32, kind="ExternalInput")
    bhgTs_d = nc.dram_tensor("bhgTs", [P, nct], F32, kind="ExternalInput")
    out_d = nc.dram_tensor("out", [nmy, dim], BF16, kind="ExternalOutput")

    with tile.TileContext(nc) as tc, ExitStack() as st:
        constp = st.enter_context(tc.tile_pool(name="const", bufs=1))
        psump = st.enter_context(tc.tile_pool(name="psum", bufs=8, space="PSUM"))
        dramp = st.enter_context(tc.tile_pool(name="dram", bufs=1, space="DRAM"))
        mainp = st.enter_context(tc.tile_pool(name="main", bufs=1))

        # v AllGather bounce buffers, one per 512-col slab of the own half
        vgi = [dramp.tile([P, njt, CC], F8, tag=f"vgi{k}", name=f"vgi{k}")
               for k in range(n_cc)]
        vgo = [dramp.tile([2, P, njt, CC], F8, tag=f"vgo{k}", name=f"vgo{k}")
               for k in range(n_cc)]

        # ---- constants ----
        wqk8_sb = constp.tile([P, nd2, 2, e], F8, tag="wqk8")
        nc.sync.dma_start(wqk8_sb[:], wqk8_d[:])
        bqk_sb = constp.tile([e, 1], F32, tag="bqk")
        nc.sync.dma_start(bqk_sb[:], bqk_d[:])
        gq_sb = constp.tile([e, 1], F32, tag="gq")
        nc.sync.dma_start(gq_sb[:], gq_d[:])
        bq_sb = constp.tile([e, 1], F32, tag="bq")
        nc.sync.dma_start(bq_sb[:], bq_d[:])
        gk_sb = constp.tile([e, 1], F32, tag="gk")
        nc.sync.dma_start(gk_sb[:], gk_d[:])
        bk_sb = constp.tile([e, 1], F32, tag="bk")
        nc.sync.dma_start(bk_sb[:], bk_d[:])
        bhg_sb = constp.tile([P, nct], F32, tag="bhg")
        nc.sync.dma_start(bhg_sb[:], bhgT_d[:])
        bhv_sb = constp.tile([1, hh], F32, tag="bhv")
        nc.sync.dma_start(bhv_sb[:], bhv_d[:])
        ones_sb = constp.tile([1, P], F32, tag="ones")
        nc.vector.memset(ones_sb[:], 1.0)

        # tiny AllGather to warm the collective stream (first real op
        # otherwise pays a ~50us cold-start)
        warm_in = dramp.tile([1, 64], F32, tag="warm_in", name="warm_in")
        warm_out = dramp.tile([2, 64], F32, tag="warm_out", name="warm_out")
        warm_sb = constp.tile([1, 64], F32, tag="warm_sb")
        nc.vector.memset(warm_sb[:], 0.0)
        nc.gpsimd.dma_start(warm_in[:], warm_sb[:])
        nc.gpsimd.collective_compute("AllGather", ALU.bypass,
                                     replica_groups=pairs,
                                     ins=[warm_in.opt()],
                                     outs=[warm_out.opt()])

        # persistent activations
        qT_sb = mainp.tile([e, nmy], BF16, tag="qT", name="qT")
        kT_sb = mainp.tile([e, seq], BF16, tag="kT", name="kT")
        v8_sb = mainp.tile([P, njt, H], F8, tag="v8", name="v8")
        gate8_sb = mainp.tile([P, nct, nmy], F8, tag="gate8", name="gate8")
        at8_sb = mainp.tile([P, njt, IC], F8, tag="at8", name="at8")

        def scores(c, pool):
            # attnT[j, own chunk c] = relu(sim*sqrt(sa)/seq)^2 -> fp8
            chunk = slice(c * IC, (c + 1) * IC)
            for jt in range(njt):
                ps = psump.tile([P, IC], F32, tag="ps", name="ps")
                nc.tensor.matmul(ps[:], kT_sb[:, jt * P:(jt + 1) * P],
                                 qT_sb[:, chunk], start=True, stop=True)
                rstage = pool.tile([P, IC], BF16, tag="rstage", bufs=4,
                                   name="rstage")
                nc.scalar.activation(rstage[:], ps[:], AF.Relu, scale=RSC)
                nc.vector.tensor_tensor(at8_sb[:, jt, :], rstage[:],
                                        rstage[:], ALU.mult)

        with tc.tile_pool(name="ph1", bufs=1) as ph1p, \
             tc.tile_pool(name="wstream", bufs=1) as wsp, \
             tc.tile_pool(name="qkp", bufs=1) as qkp:
            x8g_sb = ph1p.tile([P, nd2, 2, seq], F8, tag="x8g", name="x8g")
            for quar in range(4):
                cols = slice(quar * (seq // 4), (quar + 1) * (seq // 4))
                nc.sync.dma_start(x8g_sb[:, :, :, cols], x8g_d[:, :, :, cols])
            x8m_sb = ph1p.tile([P, nd2, 2, nmy], F8, tag="x8m", name="x8m")
            nc.sync.dma_start(x8m_sb[:], x8my_d[:])

            # ---- qk projection (fp8 DoubleRow) ----
            # kT from the global-order x, qT from the own-rows x.
            # silu(u) = u * sigmoid(u); no Silu LUT in the interp, so build
            # from Sigmoid (ACT) + mult (DVE).
            qkg_sb = qkp.tile([e, seq], BF16, tag="qkg", name="qkg")
            qkm_sb = qkp.tile([e, nmy], BF16, tag="qkm", name="qkm")

            def qk_proj(n_chunks, xsb, qk_sb):
                for ic in range(n_chunks):
                    cols = slice(ic * IC, (ic + 1) * IC)
                    ps = psump.tile([e, IC], F32, tag="ps", name="ps")
                    for dp in range(nd2):
                        nc.tensor.matmul(ps[:], wqk8_sb[:, dp],
                                         xsb[:, dp, :, cols],
                                         start=(dp == 0),
                                         stop=(dp == nd2 - 1),
                                         perf_mode=DR)
                    nc.scalar.activation(qk_sb[:, cols], ps[:], AF.Silu,
                                         bias=bqk_sb[:], scale=INV_H)

            qk_proj(n_ic, x8g_sb, qkg_sb)
            nc.vector.tensor_scalar(kT_sb[:], qkg_sb[:], gk_sb[:],
                                    bk_sb[:], ALU.mult, ALU.add)

            # ---- v half: v[j, c], exchanged via pairwise AllGather ----
            # the AG output is indexed by source rank = hidden-half index,
            # so assembling the full v is rank-independent; the per-slab
            # pipeline hides the exchange behind the gate projection below
            for cc in range(n_cc):
                ccs = slice(cc * CC, (cc + 1) * CC)
                wv8 = wsp.tile([P, nd2, 2, CC], F8, tag="wv8", bufs=2,
                               name="wv8")
                nc.sync.dma_start(wv8[:], whv8_d[cc])
                for jt in range(njt):
                    jts = slice(jt * P, (jt + 1) * P)
                    ps = psump.tile([P, CC], F32, tag="ps", name="ps")
                    for dp in range(nd2):
                        nc.tensor.matmul(ps[:], x8g_sb[:, dp, :, jts],
                                         wv8[:, dp], start=(dp == 0),
                                         stop=(not with_bhv and dp == nd2 - 1),
                                         perf_mode=DR)
                    if with_bhv:
                        nc.tensor.matmul(ps[:], ones_sb[:], bhv_sb[:, ccs],
                                         start=False, stop=True)
                    vs = wsp.tile([P, CC], BF16, tag="vs", bufs=2, name="vs")
                    nc.scalar.activation(vs[:], ps[:], AF.Silu, scale=INV_H)
                    vst = wsp.tile([P, CC], F8, tag="vst", bufs=3, name="vst")
                    nc.vector.tensor_scalar(vst[:], vs[:], SV, None, ALU.mult)
                    nc.scalar.dma_start(vgi[cc][:, jt, :], vst[:])
                nc.gpsimd.collective_compute("AllGather", ALU.bypass,
                                             replica_groups=pairs,
                                             ins=[vgi[cc].opt()],
                                             outs=[vgo[cc].opt()])
                for s in range(2):
                    c0 = s * hh + cc * CC
                    nc.gpsimd.dma_start(v8_sb[:, :, c0:c0 + CC], vgo[cc][s])

            qk_proj(n_mc, x8m_sb, qkm_sb)
            nc.vector.tensor_scalar(qT_sb[:], qkm_sb[:], gq_sb[:],
                                    bq_sb[:], ALU.mult, ALU.add)
            # scores for own chunk 0: the relu/square hide under the gate
            # projection's matmuls
            scores(0, wsp)

            # ---- gate: gateT[c, i] over the FULL hidden, own rows only ----
            for ct in range(nct):
                cts = slice(ct * P, (ct + 1) * P)
                wg8 = wsp.tile([P, nd2, 2, P], F8, tag="wg8", bufs=2,
                               name="wg8")
                nc.sync.dma_start(wg8[:], whg8_d[ct])
                for ic in range(n_mc):
                    cols = slice(ic * IC, (ic + 1) * IC)
                    ps = psump.tile([P, IC], F32, tag="ps", name="ps")
                    for dp in range(nd2):
                        nc.tensor.matmul(ps[:], wg8[:, dp],
                                         x8m_sb[:, dp, :, cols],
                                         start=(dp == 0), stop=(dp == nd2 - 1),
                                         perf_mode=DR)
                    gs = wsp.tile([P, IC], BF16, tag="gs", bufs=2, name="gs")
                    nc.scalar.activation(gs[:], ps[:], AF.Silu,
                                         bias=bhg_sb[:, ct:ct + 1],
                                         scale=INV_H)
                    nc.vector.tensor_scalar(gate8_sb[:, ct, cols], gs[:],
                                            KGF, None, ALU.mult)

        # ---- attention + output over own rows ----
        with tc.tile_pool(name="ph2", bufs=1) as ph2p:
            og8_sb = ph2p.tile([P, nct, nmy], F8, tag="og8", name="og8")
            # prefetch the first two W_out slabs during the og matmuls
            wo8 = [ph2p.tile([P, ncp, 2, DC], F8, tag=f"wo8{i}",
                             name=f"wo8{i}") for i in range(2)]
            nc.sync.dma_start(wo8[0][:], wout8_d[0])
            nc.sync.dma_start(wo8[1][:], wout8_d[1])

            def og(c):
                # ogT[c over FULL hidden, own chunk] = (v^T attnT) * gate
                chunk = slice(c * IC, (c + 1) * IC)
                for ct in range(nct):
                    cts = slice(ct * P, (ct + 1) * P)
                    ps = psump.tile([P, IC], F32, tag="ps", name="ps")
                    for jp in range(njp):
                        nc.tensor.matmul(ps[:],
                                         v8_sb[:, 2 * jp:2 * jp + 2, cts],
                                         at8_sb[:, 2 * jp:2 * jp + 2, :],
                                         start=(jp == 0), stop=(jp == njp - 1),
                                         perf_mode=DR)
                    t2 = ph2p.tile([P, IC], BF16, tag="t2", bufs=2, name="t2")
                    nc.vector.tensor_scalar(t2[:], ps[:], OG_SC, None,
                                            ALU.mult)
                    nc.vector.tensor_tensor(og8_sb[:, ct, chunk], t2[:],
                                            gate8_sb[:, ct, chunk], ALU.mult)

            og(0)
            scores(1, ph2p)   # relu/square hide under og(0)'s matmuls
            og(1)
            # branch[own rows, :] = ogT^T @ Wout (full 4096 contraction),
            # fp8 W_out streamed per 512-col output slab, double-buffered
            # two slabs ahead, straight to the output
            for dc in range(n_dc):
                dcs = slice(dc * DC, (dc + 1) * DC)
                w = wo8[dc % 2]
                for it in range(nmy // P):
                    its = slice(it * P, (it + 1) * P)
                    ps = psump.tile([P, DC], F32, tag="ps", name="ps")
                    for cp in range(ncp):
                        nc.tensor.matmul(ps[:],
                                         og8_sb[:, 2 * cp:2 * cp + 2, its],
                                         w[:, cp, :, :],
                                         start=(cp == 0),
                                         stop=(cp == ncp - 1),
                                         perf_mode=DR)
                    po = ph2p.tile([P, DC], BF16, tag="po", bufs=4, name="po")
                    nc.vector.tensor_scalar(po[:], ps[:], INV_O, None,
                                            ALU.mult)
                    nc.scalar.dma_start(out_d[it * P:(it + 1) * P, dcs],
                                        po[:])
                if dc + 2 < n_dc:
                    nc.sync.dma_start(w[:], wout8_d[dc + 2])

    nc.compile()
    return nc


def own_rows(seq, h):
    """Rows owned by pair-member h: contiguous half of the sequence."""
    nmy = seq // 2
    return np.arange(h * nmy, (h + 1) * nmy)


def _q8(a, s):
    """Quantize a*s to fp8 e4m3, clipping to +-240 (TRN e4m3 infs at 256)."""
    return np.clip(a.astype(np.float32) * s, -F8MAX, F8MAX).astype(
        ml_dtypes.float8_e4m3)


def _dr_pack(m8):
    """[K, F] fp8 -> [128, K//256, 2, F] DoubleRow pair layout."""
    K, F = m8.shape
    return np.ascontiguousarray(
        m8.reshape(K // 256, 2, P, F).transpose(2, 0, 1, 3))


def make_in_maps(x, W_hidden, b_hidden, W_qk, b_qk, gamma_q, beta_q,
                 gamma_k, beta_k, W_out, b_out, n_cores=8):
    """Host-side quantization/layout prep.  Returns per-core input dicts."""
    B, seq, dim = x.shape
    H2 = W_hidden.shape[1]
    H = H2 // 2
    hh = H // 2
    nct = H // P
    nmy = seq // 2
    f32 = np.float32
    x8_cache = {}
    xm_cache = {}
    half_cache = {}
    # full-hidden tensors, shared by both pair members
    _wg = _dr_pack(_q8(W_hidden[:, H:], SW))
    whg8 = np.ascontiguousarray(np.stack(
        [_wg[:, :, :, c0:c0 + P] for c0 in range(0, H, P)]))
    _wo = _dr_pack(_q8(W_out, SWO))
    wout8 = np.ascontiguousarray(np.stack(
        [_wo[:, :, :, dc * 512:(dc + 1) * 512] for dc in range(dim // 512)]))
    wqk8 = _dr_pack(_q8(W_qk, SW))
    bhgT = np.ascontiguousarray(b_hidden[H:].astype(f32).reshape(nct, P).T)

    def halves(h):
        if h not in half_cache:
            cs = slice(h * hh, (h + 1) * hh)
            half_cache[h] = {
                "whv8": np.ascontiguousarray(np.stack(
                    [_dr_pack(_q8(W_hidden[:, cs], SW))[:, :, :, c0:c0 + 512]
                     for c0 in range(0, hh, 512)])),
                "bhv": (b_hidden[cs].astype(f32) * (SX * SW)).reshape(1, -1),
            }
        return half_cache[h]

    in_maps = []
    for core in range(n_cores):
        b, h = core // 2, core % 2
        if b not in x8_cache:
            x8_cache[b] = _dr_pack(_q8(np.ascontiguousarray(x[b].T), SX))
        if (b, h) not in xm_cache:
            xm_cache[(b, h)] = np.ascontiguousarray(
                x8_cache[b][:, :, :, h * nmy:(h + 1) * nmy])
        hv = halves(h)
        in_maps.append({
            "x8g": x8_cache[b],
            "x8my": xm_cache[(b, h)],
            "whv8": hv["whv8"],
            "whg8": whg8,
            "wout8": wout8,
            "wqk8": wqk8,
            "bqk": b_qk.reshape(-1, 1).astype(f32),
            "bqks": (b_qk.reshape(-1, 1).astype(f32) * (SX * SW)),
            "gq": gamma_q.reshape(-1, 1).astype(f32),
            "bq": beta_q.reshape(-1, 1).astype(f32),
            "gk": gamma_k.reshape(-1, 1).astype(f32),
            "bk": beta_k.reshape(-1, 1).astype(f32),
            "bhv": hv["bhv"],
            "bhgT": bhgT,
            "bhgTs": bhgT * (SX * SW),
        })
    return in_maps


_NC_CACHE = {}


def _get_nc(seq, dim, hh, n_cores, with_bhv=True):
    key = (seq, dim, hh, n_cores, with_bhv)
    if key not in _NC_CACHE:
        _NC_CACHE[key] = build_gau_nc(seq=seq, dim=dim, hh=hh,
                                      n_cores=n_cores, with_bhv=with_bhv)
    return _NC_CACHE[key]


def kernel(x, W_hidden, b_hidden, W_qk, b_qk, gamma_q, beta_q, gamma_k,
           beta_k, W_out, b_out):
    x = np.asarray(x)
    B, seq, dim = x.shape
    hh = W_hidden.shape[1] // 4
    n_cores = 2 * B
    with_bhv = bool(np.any(np.asarray(b_hidden)[: 2 * hh] != 0))
    nc = _get_nc(seq, dim, hh, n_cores, with_bhv=with_bhv)
    in_maps = make_in_maps(x, np.asarray(W_hidden), np.asarray(b_hidden),
                           np.asarray(W_qk), np.asarray(b_qk),
                           np.asarray(gamma_q), np.asarray(beta_q),
                           np.asarray(gamma_k), np.asarray(beta_k),
                           np.asarray(W_out), np.asarray(b_out),
                           n_cores=n_cores)
    res = run_bass_kernel_spmd(nc, in_maps, core_ids=list(range(n_cores)))
    bo = np.asarray(b_out).astype(np.float32)
    out = np.empty((B, seq, dim), np.float32)
    for b in range(B):
        for h in range(2):
            rows = own_rows(seq, h)
            out[b, rows] = (res.results[2 * b + h]["out"].astype(np.float32)
                            + x[b][rows] + bo)
    return out


# revision 19
# speedup vs baseline: 1.1604x; 1.0236x over previous
"""GAU (Gated Attention Unit, relu^2 attention) Trainium2 Bass kernel, fp8.

Problem shapes: x [4, 2048, 2048] f32; W_hidden [2048, 8192]; W_qk [2048, 128];
W_out [4096, 2048]; out = GAU(x) + x.

Sharding (8 cores): core = 2*batch + h, h in {0,1}.  Each pair of cores
handles one batch.  The VALUE/GATE hidden projections are column-split in
half within the pair (core h computes v for hidden cols
[h*2048:(h+1)*2048]); the computed v half is then exchanged with a
pairwise AllGather (fp8, 4MB, pipelined per 512-column slab behind the
gate projection) so both cores hold the full [2048, 4096] v.  An
AllGather works here precisely because both cores want the SAME full v
-- its output is indexed by source rank, which equals the hidden-half
index, keeping the program rank-independent.

Everything downstream is split by QUERY rows instead: core h owns the
global rows [h*1024, (h+1)*1024) and computes scores, attn, gate, og and
the output projection (full 4096-deep contraction, W_out streamed fp8)
only for those rows, writing final bf16 branch rows straight to its
output.  The row split is baked in on the host: a second input tensor
x8my carries only the core's own 1024 x columns for the q projection and
the gate's moving operand, while x8g (global order) feeds k, and v.
There is no partial-sum ReduceScatter and no serial collective tail.
The host adds the residual x and b_out in fp32 (the branch is ~0.3% of
the output norm, so a bf16 branch costs ~3e-6 relative error).

All projections run as fp8e4 (e4m3, max +-240) DoubleRow matmuls: both
operands are packed [128, 2, free] pairing two 128-row contraction
tiles, giving 2 fp8 MACs/PE-cell/cycle.  Quantization scales are powers
of two, chosen so every fp8 tensor's max is >=2.7x below 240 (measured
on the real data distribution; host clips before casting to avoid
e4m3's non-saturating inf at 256):

  x*16, W_hidden*512, W_qk*512, W_out*512, v*16, attn*2^14, gate*32,
  og*1024

PSUM stays fp32; descales fold into the activation `scale` ports and DVE
tensor_scalar immediates.  silu runs on the scalar engine's native Silu
LUT (one ACT op; the CoreSim interp lacks the LUT, so this kernel is
hardware-only).  qT/kT and the relu^2 scores matmul stay bf16 (K=128,
cheap; scores are NOT pair-replicated since each core only needs its own
rows).  The scores for chunk 0 are emitted between v and gate, and for
chunk 1 between og(0) and og(1), hiding their scalar-engine relu under
matmul-heavy phases.  W_out streams per 512-column output slab from a
slab-contiguous DRAM layout, double-buffered two slabs ahead.
"""

import numpy as np
import ml_dtypes
from contextlib import ExitStack

import concourse.bass as bass
import concourse.bacc as bacc
import concourse.mybir as mybir
import concourse.tile as tile
from concourse.bass_utils import run_bass_kernel_spmd

BF16 = mybir.dt.bfloat16
F32 = mybir.dt.float32
F8 = mybir.dt.float8e4
AF = mybir.ActivationFunctionType
ALU = mybir.AluOpType
DR = mybir.MatmulPerfMode.DoubleRow
P = 128
F8MAX = 240.0

# quantization scales (powers of two; see module docstring)
SX = 16.0       # x
SW = 512.0      # W_hidden (both halves), W_qk
SWO = 512.0     # W_out
SV = 16.0       # v activations
SA = 16384.0    # attn = relu(sim)^2    (sqrt(SA) = 128 folds into the relu)
KGF = 32.0      # gate activations
SO = 1024.0     # og activations

INV_H = 1.0 / (SX * SW)        # hidden/qk psum -> real pre-activation
T_V = SV / (SX * SW)           # v-path u multiplier
T_G = KGF / (SX * SW)          # gate-path u multiplier
OG_SC = SO / (SA * SV * KGF)   # og = (psum_av * OG_SC) * gate8
RSC = 128.0 / 2048.0           # sqrt(SA)/seq relu scale
INV_O = 1.0 / (SO * SWO)       # out psum -> real branch rows


def build_gau_nc(seq=2048, dim=2048, hh=2048, n_cores=8, with_bhv=True):
    """Build the SPMD program.  hh = per-core v half width."""
    e = P
    nd2 = dim // (2 * P)   # DoubleRow contraction pair tiles
    njt = seq // P
    njp = njt // 2
    H = 2 * hh             # full hidden width
    nct = H // P           # full-hidden c tiles (gate, og)
    ncp = nct // 2
    nmy = seq // 2         # own query rows per core
    IC = 512
    n_ic = seq // IC
    n_mc = nmy // IC       # own-row chunks
    CC = 512
    n_cc = hh // CC
    DC = 512
    n_dc = dim // DC
    pairs = [[2 * g, 2 * g + 1] for g in range(n_cores // 2)]

    nc = bacc.Bacc("TRN2", target_bir_lowering=False, debug=False,
                   num_devices=n_cores)

    x8g_d = nc.dram_tensor("x8g", [P, nd2, 2, seq], F8, kind="ExternalInput")
    x8my_d = nc.dram_tensor("x8my", [P, nd2, 2, nmy], F8,
                            kind="ExternalInput")
    whv8_d = nc.dram_tensor("whv8", [n_cc, P, nd2, 2, CC], F8,
                            kind="ExternalInput")
    whg8_d = nc.dram_tensor("whg8", [nct, P, nd2, 2, P], F8,
                            kind="ExternalInput")
    wqk8_d = nc.dram_tensor("wqk8", [P, nd2, 2, e], F8, kind="ExternalInput")
    wout8_d = nc.dram_tensor("wout8", [n_dc, P, ncp, 2, DC], F8,
                             kind="ExternalInput")
    bqk_d = nc.dram_tensor("bqk", [e, 1], F32, kind="ExternalInput")
    bqks_d = nc.dram_tensor("bqks", [e, 1], F32, kind="ExternalInput")
    gq_d = nc.dram_tensor("gq", [e, 1], F32, kind="ExternalInput")
    bq_d = nc.dram_tensor("bq", [e, 1], F32, kind="ExternalInput")
    gk_d = nc.dram_tensor("gk", [e, 1], F32, kind="ExternalInput")
    bk_d = nc.dram_tensor("bk", [e, 1], F32, kind="ExternalInput")
    bhv_d = nc.dram_tensor("bhv", [1, hh], F32, kind="ExternalInput")
    bhgT_d = nc.dram_tensor("bhgT", [P, nct], F32, kind="ExternalInput")
    bhgTs_d = nc.dram_tensor("bhgTs", [P, nct], F32, kind="ExternalInput")
    out_d = nc.dram_tensor("out", [nmy, dim], BF16, kind="ExternalOutput")

    with tile.TileContext(nc) as tc, ExitStack() as st:
        constp = st.enter_context(tc.tile_pool(name="const", bufs=1))
        psump = st.enter_context(tc.tile_pool(name="psum", bufs=8, space="PSUM"))
        dramp = st.enter_context(tc.tile_pool(name="dram", bufs=1, space="DRAM"))
        mainp = st.enter_context(tc.tile_pool(name="main", bufs=1))

        # v AllGather bounce buffers, one per 512-col slab of the own half
        vgi = [dramp.tile([P, njt, CC], F8, tag=f"vgi{k}", name=f"vgi{k}")
               for k in range(n_cc)]
        vgo = [dramp.tile([2, P, njt, CC], F8, tag=f"vgo{k}", name=f"vgo{k}")
               for k in range(n_cc)]

        # ---- constants ----
        wqk8_sb = constp.tile([P, nd2, 2, e], F8, tag="wqk8")
        nc.sync.dma_start(wqk8_sb[:], wqk8_d[:])
        bqk_sb = constp.tile([e, 1], F32, tag="bqk")
        nc.sync.dma_start(bqk_sb[:], bqk_d[:])
        gq_sb = constp.tile([e, 1], F32, tag="gq")
        nc.sync.dma_start(gq_sb[:], gq_d[:])
        bq_sb = constp.tile([e, 1], F32, tag="bq")
        nc.sync.dma_start(bq_sb[:], bq_d[:])
        gk_sb = constp.tile([e, 1], F32, tag="gk")
        nc.sync.dma_start(gk_sb[:], gk_d[:])
        bk_sb = constp.tile([e, 1], F32, tag="bk")
        nc.sync.dma_start(bk_sb[:], bk_d[:])
        bhg_sb = constp.tile([P, nct], F32, tag="bhg")
        nc.sync.dma_start(bhg_sb[:], bhgT_d[:])
        bhv_sb = constp.tile([1, hh], F32, tag="bhv")
        nc.sync.dma_start(bhv_sb[:], bhv_d[:])
        ones_sb = constp.tile([1, P], F32, tag="ones")
        nc.vector.memset(ones_sb[:], 1.0)

        # tiny AllGather to warm the collective stream (first real op
        # otherwise pays a ~50us cold-start)
        warm_in = dramp.tile([1, 64], F32, tag="warm_in", name="warm_in")
        warm_out = dramp.tile([2, 64], F32, tag="warm_out", name="warm_out")
        warm_sb = constp.tile([1, 64], F32, tag="warm_sb")
        nc.vector.memset(warm_sb[:], 0.0)
        nc.gpsimd.dma_start(warm_in[:], warm_sb[:])
        nc.gpsimd.collective_compute("AllGather", ALU.bypass,
                                     replica_groups=pairs,
                                     ins=[warm_in.opt()],
                                     outs=[warm_out.opt()])

        # persistent activations
        qT_sb = mainp.tile([e, nmy], BF16, tag="qT", name="qT")
        kT_sb = mainp.tile([e, seq], BF16, tag="kT", name="kT")
        v8_sb = mainp.tile([P, njt, H], F8, tag="v8", name="v8")
        gate8_sb = mainp.tile([P, nct, nmy], F8, tag="gate8", name="gate8")
        at8_sb = mainp.tile([P, njt, IC], F8, tag="at8", name="at8")

        def scores(c, pool):
            # attnT[j, own chunk c] = relu(sim*sqrt(sa)/seq)^2 -> fp8
            chunk = slice(c * IC, (c + 1) * IC)
            for jt in range(njt):
                ps = psump.tile([P, IC], F32, tag="ps", name="ps")
                nc.tensor.matmul(ps[:], kT_sb[:, jt * P:(jt + 1) * P],
                                 qT_sb[:, chunk], start=True, stop=True)
                rstage = pool.tile([P, IC], BF16, tag="rstage", bufs=4,
                                   name="rstage")
                nc.scalar.activation(rstage[:], ps[:], AF.Relu, scale=RSC)
                nc.vector.tensor_tensor(at8_sb[:, jt, :], rstage[:],
                                        rstage[:], ALU.mult)

        with tc.tile_pool(name="ph1", bufs=1) as ph1p, \
             tc.tile_pool(name="wstream", bufs=1) as wsp, \
             tc.tile_pool(name="qkp", bufs=1) as qkp:
            x8g_sb = ph1p.tile([P, nd2, 2, seq], F8, tag="x8g", name="x8g")
            for quar in range(4):
                cols = slice(quar * (seq // 4), (quar + 1) * (seq // 4))
                nc.sync.dma_start(x8g_sb[:, :, :, cols], x8g_d[:, :, :, cols])
            x8m_sb = ph1p.tile([P, nd2, 2, nmy], F8, tag="x8m", name="x8m")
            nc.scalar.dma_start(x8m_sb[:], x8my_d[:])

            # ---- qk projection (fp8 DoubleRow) ----
            # kT from the global-order x, qT from the own-rows x.
            # silu(u) = u * sigmoid(u); no Silu LUT in the interp, so build
            # from Sigmoid (ACT) + mult (DVE).
            qkg_sb = qkp.tile([e, seq], BF16, tag="qkg", name="qkg")
            qkm_sb = qkp.tile([e, nmy], BF16, tag="qkm", name="qkm")

            def qk_proj(n_chunks, xsb, qk_sb):
                for ic in range(n_chunks):
                    cols = slice(ic * IC, (ic + 1) * IC)
                    ps = psump.tile([e, IC], F32, tag="ps", name="ps")
                    for dp in range(nd2):
                        nc.tensor.matmul(ps[:], wqk8_sb[:, dp],
                                         xsb[:, dp, :, cols],
                                         start=(dp == 0),
                                         stop=(dp == nd2 - 1),
                                         perf_mode=DR)
                    nc.scalar.activation(qk_sb[:, cols], ps[:], AF.Silu,
                                         bias=bqk_sb[:], scale=INV_H)

            wv8 = [wsp.tile([P, nd2, 2, CC], F8, tag=f"wv8{i}",
                            name=f"wv8{i}") for i in range(2)]
            nc.scalar.dma_start(wv8[0][:], whv8_d[0])
            nc.scalar.dma_start(wv8[1][:], whv8_d[1])

            qk_proj(n_ic, x8g_sb, qkg_sb)
            nc.vector.tensor_scalar(kT_sb[:], qkg_sb[:], gk_sb[:],
                                    bk_sb[:], ALU.mult, ALU.add)

            # ---- v half: v[j, c], exchanged via pairwise AllGather ----
            # the AG output is indexed by source rank = hidden-half index,
            # so assembling the full v is rank-independent; the per-slab
            # pipeline hides the exchange behind the gate projection below
            for cc in range(n_cc):
                ccs = slice(cc * CC, (cc + 1) * CC)
                wv = wv8[cc % 2]
                for jt in range(njt):
                    jts = slice(jt * P, (jt + 1) * P)
                    ps = psump.tile([P, CC], F32, tag="ps", name="ps")
                    for dp in range(nd2):
                        nc.tensor.matmul(ps[:], x8g_sb[:, dp, :, jts],
                                         wv[:, dp], start=(dp == 0),
                                         stop=(not with_bhv and dp == nd2 - 1),
                                         perf_mode=DR)
                    if with_bhv:
                        nc.tensor.matmul(ps[:], ones_sb[:], bhv_sb[:, ccs],
                                         start=False, stop=True)
                    vs = wsp.tile([P, CC], BF16, tag="vs", bufs=2, name="vs")
                    nc.scalar.activation(vs[:], ps[:], AF.Silu, scale=INV_H)
                    vst = wsp.tile([P, CC], F8, tag="vst", bufs=3, name="vst")
                    nc.vector.tensor_scalar(vst[:], vs[:], SV, None, ALU.mult)
                    nc.scalar.dma_start(vgi[cc][:, jt, :], vst[:])
                nc.gpsimd.collective_compute("AllGather", ALU.bypass,
                                             replica_groups=pairs,
                                             ins=[vgi[cc].opt()],
                                             outs=[vgo[cc].opt()])
                for s in range(2):
                    c0 = s * hh + cc * CC
                    nc.gpsimd.dma_start(v8_sb[:, :, c0:c0 + CC], vgo[cc][s])
                if cc + 2 < n_cc:
                    nc.scalar.dma_start(wv[:], whv8_d[cc + 2])

            wg8 = [wsp.tile([P, nd2, 2, P], F8, tag=f"wg8{i}",
                            name=f"wg8{i}") for i in range(2)]
            nc.scalar.dma_start(wg8[0][:], whg8_d[0])
            nc.scalar.dma_start(wg8[1][:], whg8_d[1])

            qk_proj(n_mc, x8m_sb, qkm_sb)
            nc.vector.tensor_scalar(qT_sb[:], qkm_sb[:], gq_sb[:],
                                    bq_sb[:], ALU.mult, ALU.add)
            # scores for own chunk 0: the relu/square hide under the gate
            # projection's matmuls
            scores(0, wsp)

            # ---- gate: gateT[c, i] over the FULL hidden, own rows only ----
            for ct in range(nct):
                cts = slice(ct * P, (ct + 1) * P)
                wg = wg8[ct % 2]
                for ic in range(n_mc):
                    cols = slice(ic * IC, (ic + 1) * IC)
                    ps = psump.tile([P, IC], F32, tag="ps", name="ps")
                    for dp in range(nd2):
                        nc.tensor.matmul(ps[:], wg[:, dp],
                                         x8m_sb[:, dp, :, cols],
                                         start=(dp == 0), stop=(dp == nd2 - 1),
                                         perf_mode=DR)
                    gs = wsp.tile([P, IC], BF16, tag="gs", bufs=2, name="gs")
                    nc.scalar.activation(gs[:], ps[:], AF.Silu,
                                         bias=bhg_sb[:, ct:ct + 1],
                                         scale=INV_H)
                    nc.vector.tensor_scalar(gate8_sb[:, ct, cols], gs[:],
                                            KGF, None, ALU.mult)
                if ct + 2 < nct:
                    nc.scalar.dma_start(wg[:], whg8_d[ct + 2])

        # ---- attention + output over own rows ----
        with tc.tile_pool(name="ph2", bufs=1) as ph2p:
            og8_sb = ph2p.tile([P, nct, nmy], F8, tag="og8", name="og8")
            # prefetch the first two W_out slabs during the og matmuls
            wo8 = [ph2p.tile([P, ncp, 2, DC], F8, tag=f"wo8{i}",
                             name=f"wo8{i}") for i in range(2)]
            nc.sync.dma_start(wo8[0][:], wout8_d[0])
            nc.sync.dma_start(wo8[1][:], wout8_d[1])

            def og(c):
                # ogT[c over FULL hidden, own chunk] = (v^T attnT) * gate
                chunk = slice(c * IC, (c + 1) * IC)
                for ct in range(nct):
                    cts = slice(ct * P, (ct + 1) * P)
                    ps = psump.tile([P, IC], F32, tag="ps", name="ps")
                    for jp in range(njp):
                        nc.tensor.matmul(ps[:],
                                         v8_sb[:, 2 * jp:2 * jp + 2, cts],
                                         at8_sb[:, 2 * jp:2 * jp + 2, :],
                                         start=(jp == 0), stop=(jp == njp - 1),
                                         perf_mode=DR)
                    t2 = ph2p.tile([P, IC], BF16, tag="t2", bufs=2, name="t2")
                    nc.vector.tensor_scalar(t2[:], ps[:], OG_SC, None,
                                            ALU.mult)
                    nc.vector.tensor_tensor(og8_sb[:, ct, chunk], t2[:],
                                            gate8_sb[:, ct, chunk], ALU.mult)

            og(0)
            scores(1, ph2p)   # relu/square hide under og(0)'s matmuls
            og(1)
            # branch[own rows, :] = ogT^T @ Wout (full 4096 contraction),
            # fp8 W_out streamed per 512-col output slab, double-buffered
            # two slabs ahead, straight to the output
            for dc in range(n_dc):
                dcs = slice(dc * DC, (dc + 1) * DC)
                w = wo8[dc % 2]
                for it in range(nmy // P):
                    its = slice(it * P, (it + 1) * P)
                    ps = psump.tile([P, DC], F32, tag="ps", name="ps")
                    for cp in range(ncp):
                        nc.tensor.matmul(ps[:],
                                         og8_sb[:, 2 * cp:2 * cp + 2, its],
                                         w[:, cp, :, :],
                                         start=(cp == 0),
                                         stop=(cp == ncp - 1),
                                         perf_mode=DR)
                    po = ph2p.tile([P, DC], BF16, tag="po", bufs=4, name="po")
                    nc.vector.tensor_scalar(po[:], ps[:], INV_O, None,
                                            ALU.mult)
                    nc.scalar.dma_start(out_d[it * P:(it + 1) * P, dcs],
                                        po[:])
                if dc + 2 < n_dc:
                    nc.sync.dma_start(w[:], wout8_d[dc + 2])

    nc.compile()
    return nc


def own_rows(seq, h):
    """Rows owned by pair-member h: contiguous half of the sequence."""
    nmy = seq // 2
    return np.arange(h * nmy, (h + 1) * nmy)


def _q8(a, s):
    """Quantize a*s to fp8 e4m3, clipping to +-240 (TRN e4m3 infs at 256)."""
    return np.clip(a.astype(np.float32) * s, -F8MAX, F8MAX).astype(
        ml_dtypes.float8_e4m3)


def _dr_pack(m8):
    """[K, F] fp8 -> [128, K//256, 2, F] DoubleRow pair layout."""
    K, F = m8.shape
    return np.ascontiguousarray(
        m8.reshape(K // 256, 2, P, F).transpose(2, 0, 1, 3))


def make_in_maps(x, W_hidden, b_hidden, W_qk, b_qk, gamma_q, beta_q,
                 gamma_k, beta_k, W_out, b_out, n_cores=8):
    """Host-side quantization/layout prep.  Returns per-core input dicts."""
    B, seq, dim = x.shape
    H2 = W_hidden.shape[1]
    H = H2 // 2
    hh = H // 2
    nct = H // P
    nmy = seq // 2
    f32 = np.float32
    x8_cache = {}
    xm_cache = {}
    half_cache = {}
    # full-hidden tensors, shared by both pair members
    _wg = _dr_pack(_q8(W_hidden[:, H:], SW))
    whg8 = np.ascontiguousarray(np.stack(
        [_wg[:, :, :, c0:c0 + P] for c0 in range(0, H, P)]))
    _wo = _dr_pack(_q8(W_out, SWO))
    wout8 = np.ascontiguousarray(np.stack(
        [_wo[:, :, :, dc * 512:(dc + 1) * 512] for dc in range(dim // 512)]))
    wqk8 = _dr_pack(_q8(W_qk, SW))
    bhgT = np.ascontiguousarray(b_hidden[H:].astype(f32).reshape(nct, P).T)

    def halves(h):
        if h not in half_cache:
            cs = slice(h * hh, (h + 1) * hh)
            half_cache[h] = {
                "whv8": np.ascontiguousarray(np.stack(
                    [_dr_pack(_q8(W_hidden[:, cs], SW))[:, :, :, c0:c0 + 512]
                     for c0 in range(0, hh, 512)])),
                "bhv": (b_hidden[cs].astype(f32) * (SX * SW)).reshape(1, -1),
            }
        return half_cache[h]

    in_maps = []
    for core in range(n_cores):
        b, h = core // 2, core % 2
        if b not in x8_cache:
            x8_cache[b] = _dr_pack(_q8(np.ascontiguousarray(x[b].T), SX))
        if (b, h) not in xm_cache:
            xm_cache[(b, h)] = np.ascontiguousarray(
                x8_cache[b][:, :, :, h * nmy:(h + 1) * nmy])
        hv = halves(h)
        in_maps.append({
            "x8g": x8_cache[b],
            "x8my": xm_cache[(b, h)],
            "whv8": hv["whv8"],
            "whg8": whg8,
            "wout8": wout8,
            "wqk8": wqk8,
            "bqk": b_qk.reshape(-1, 1).astype(f32),
            "bqks": (b_qk.reshape(-1, 1).astype(f32) * (SX * SW)),
            "gq": gamma_q.reshape(-1, 1).astype(f32),
            "bq": beta_q.reshape(-1, 1).astype(f32),
            "gk": gamma_k.reshape(-1, 1).astype(f32),
            "bk": beta_k.reshape(-1, 1).astype(f32),
            "bhv": hv["bhv"],
            "bhgT": bhgT,
            "bhgTs": bhgT * (SX * SW),
        })
    return in_maps


_NC_CACHE = {}


def _get_nc(seq, dim, hh, n_cores, with_bhv=True):
    key = (seq, dim, hh, n_cores, with_bhv)
    if key not in _NC_CACHE:
        _NC_CACHE[key] = build_gau_nc(seq=seq, dim=dim, hh=hh,
                                      n_cores=n_cores, with_bhv=with_bhv)
    return _NC_CACHE[key]


def kernel(x, W_hidden, b_hidden, W_qk, b_qk, gamma_q, beta_q, gamma_k,
           beta_k, W_out, b_out):
    x = np.asarray(x)
    B, seq, dim = x.shape
    hh = W_hidden.shape[1] // 4
    n_cores = 2 * B
    with_bhv = bool(np.any(np.asarray(b_hidden)[: 2 * hh] != 0))
    nc = _get_nc(seq, dim, hh, n_cores, with_bhv=with_bhv)
    in_maps = make_in_maps(x, np.asarray(W_hidden), np.asarray(b_hidden),
                           np.asarray(W_qk), np.asarray(b_qk),
                           np.asarray(gamma_q), np.asarray(beta_q),
                           np.asarray(gamma_k), np.asarray(beta_k),
                           np.asarray(W_out), np.asarray(b_out),
                           n_cores=n_cores)
    res = run_bass_kernel_spmd(nc, in_maps, core_ids=list(range(n_cores)))
    bo = np.asarray(b_out).astype(np.float32)
    out = np.empty((B, seq, dim), np.float32)
    for b in range(B):
        for h in range(2):
            rows = own_rows(seq, h)
            out[b, rows] = (res.results[2 * b + h]["out"].astype(np.float32)
                            + x[b][rows] + bo)
    return out


# revision 21
# speedup vs baseline: 1.1952x; 1.0300x over previous
"""GAU (Gated Attention Unit, relu^2 attention) Trainium2 Bass kernel, fp8.

Problem shapes: x [4, 2048, 2048] f32; W_hidden [2048, 8192]; W_qk [2048, 128];
W_out [4096, 2048]; out = GAU(x) + x.

Sharding (8 cores): core = 2*batch + h, h in {0,1}.  Each pair of cores
handles one batch.  The VALUE/GATE hidden projections are column-split in
half within the pair (core h computes v for hidden cols
[h*2048:(h+1)*2048]); the computed v half is then exchanged with a
pairwise AllGather (fp8, 4MB, pipelined per 512-column slab behind the
gate projection) so both cores hold the full [2048, 4096] v.  An
AllGather works here precisely because both cores want the SAME full v
-- its output is indexed by source rank, which equals the hidden-half
index, keeping the program rank-independent.

Everything downstream is split by QUERY rows instead: core h owns the
global rows [h*1024, (h+1)*1024) and computes scores, attn, gate, og and
the output projection (full 4096-deep contraction, W_out streamed fp8)
only for those rows, writing final bf16 branch rows straight to its
output.  The row split is baked in on the host: a second input tensor
x8my carries only the core's own 1024 x columns for the q projection and
the gate's moving operand, while x8g (global order) feeds k, and v.
There is no partial-sum ReduceScatter and no serial collective tail.
The host adds the residual x and b_out in fp32 (the branch is ~0.3% of
the output norm, so a bf16 branch costs ~3e-6 relative error).

All projections run as fp8e4 (e4m3, max +-240) DoubleRow matmuls: both
operands are packed [128, 2, free] pairing two 128-row contraction
tiles, giving 2 fp8 MACs/PE-cell/cycle.  Quantization scales are powers
of two, chosen so every fp8 tensor's max is >=2.7x below 240 (measured
on the real data distribution; host clips before casting to avoid
e4m3's non-saturating inf at 256):

  x*16, W_hidden*512, W_qk*512, W_out*512, v*16, attn*2^14, gate*32,
  og*1024

PSUM stays fp32; descales fold into the activation `scale` ports and DVE
tensor_scalar immediates.  silu runs on the scalar engine's native Silu
LUT (one ACT op; the CoreSim interp lacks the LUT, so this kernel is
hardware-only).  qT/kT and the relu^2 scores matmul stay bf16 (K=128,
cheap; scores are NOT pair-replicated since each core only needs its own
rows).  The scores for chunk 0 are emitted between v and gate, and for
chunk 1 between og(0) and og(1), hiding their scalar-engine relu under
matmul-heavy phases.  W_out streams per 512-column output slab from a
slab-contiguous DRAM layout, double-buffered two slabs ahead.
"""

import numpy as np
import ml_dtypes
from contextlib import ExitStack

import concourse.bass as bass
import concourse.bacc as bacc
import concourse.mybir as mybir
import concourse.tile as tile
from concourse.bass_utils import run_bass_kernel_spmd

BF16 = mybir.dt.bfloat16
F32 = mybir.dt.float32
F8 = mybir.dt.float8e4
AF = mybir.ActivationFunctionType
ALU = mybir.AluOpType
DR = mybir.MatmulPerfMode.DoubleRow
P = 128
F8MAX = 240.0

# quantization scales (powers of two; see module docstring)
SX = 16.0       # x
SW = 512.0      # W_hidden (both halves), W_qk
SWO = 512.0     # W_out
SV = 16.0       # v activations
SA = 16384.0    # attn = relu(sim)^2    (sqrt(SA) = 128 folds into the relu)
KGF = 32.0      # gate activations
SO = 1024.0     # og activations

INV_H = 1.0 / (SX * SW)        # hidden/qk psum -> real pre-activation
T_V = SV / (SX * SW)           # v-path u multiplier
T_G = KGF / (SX * SW)          # gate-path u multiplier
OG_SC = SO / (SA * SV * KGF)   # og = (psum_av * OG_SC) * gate8
RSC = 128.0 / 2048.0           # sqrt(SA)/seq relu scale
INV_O = 1.0 / (SO * SWO)       # out psum -> real branch rows


def build_gau_nc(seq=2048, dim=2048, hh=2048, n_cores=8, with_bhv=True):
    """Build the SPMD program.  hh = per-core v half width."""
    e = P
    nd2 = dim // (2 * P)   # DoubleRow contraction pair tiles
    njt = seq // P
    njp = njt // 2
    H = 2 * hh             # full hidden width
    nct = H // P           # full-hidden c tiles (gate, og)
    ncp = nct // 2
    nmy = seq // 2         # own query rows per core
    IC = 512
    n_ic = seq // IC
    n_mc = nmy // IC       # own-row chunks
    CC = 512
    n_cc = hh // CC
    DC = 512
    n_dc = dim // DC
    pairs = [[2 * g, 2 * g + 1] for g in range(n_cores // 2)]

    nc = bacc.Bacc("TRN2", target_bir_lowering=False, debug=False,
                   num_devices=n_cores)

    x8g_d = nc.dram_tensor("x8g", [P, nd2, 2, seq], F8, kind="ExternalInput")
    x8my_d = nc.dram_tensor("x8my", [P, nd2, 2, nmy], F8,
                            kind="ExternalInput")
    whv8_d = nc.dram_tensor("whv8", [n_cc, P, nd2, 2, CC], F8,
                            kind="ExternalInput")
    whg8_d = nc.dram_tensor("whg8", [nct, P, nd2, 2, P], F8,
                            kind="ExternalInput")
    wqk8_d = nc.dram_tensor("wqk8", [P, nd2, 2, e], F8, kind="ExternalInput")
    wout8_d = nc.dram_tensor("wout8", [n_dc, P, ncp, 2, DC], F8,
                             kind="ExternalInput")
    bqk_d = nc.dram_tensor("bqk", [e, 1], F32, kind="ExternalInput")
    bqks_d = nc.dram_tensor("bqks", [e, 1], F32, kind="ExternalInput")
    gq_d = nc.dram_tensor("gq", [e, 1], F32, kind="ExternalInput")
    bq_d = nc.dram_tensor("bq", [e, 1], F32, kind="ExternalInput")
    gk_d = nc.dram_tensor("gk", [e, 1], F32, kind="ExternalInput")
    bk_d = nc.dram_tensor("bk", [e, 1], F32, kind="ExternalInput")
    bhv_d = nc.dram_tensor("bhv", [1, hh], F32, kind="ExternalInput")
    bhgT_d = nc.dram_tensor("bhgT", [P, nct], F32, kind="ExternalInput")
    bhgTs_d = nc.dram_tensor("bhgTs", [P, nct], F32, kind="ExternalInput")
    out_d = nc.dram_tensor("out", [nmy, dim], BF16, kind="ExternalOutput")

    with tile.TileContext(nc) as tc, ExitStack() as st:
        constp = st.enter_context(tc.tile_pool(name="const", bufs=1))
        psump = st.enter_context(tc.tile_pool(name="psum", bufs=8, space="PSUM"))
        dramp = st.enter_context(tc.tile_pool(name="dram", bufs=1, space="DRAM"))
        mainp = st.enter_context(tc.tile_pool(name="main", bufs=1))

        # v AllGather bounce buffers, one per 512-col slab of the own half
        vgi = [dramp.tile([P, njt, CC], F8, tag=f"vgi{k}", name=f"vgi{k}")
               for k in range(n_cc)]
        vgo = [dramp.tile([2, P, njt, CC], F8, tag=f"vgo{k}", name=f"vgo{k}")
               for k in range(n_cc)]

        # ---- constants ----
        wqk8_sb = constp.tile([P, nd2, 2, e], F8, tag="wqk8")
        nc.sync.dma_start(wqk8_sb[:], wqk8_d[:])
        bqk_sb = constp.tile([e, 1], F32, tag="bqk")
        nc.sync.dma_start(bqk_sb[:], bqk_d[:])
        gq_sb = constp.tile([e, 1], F32, tag="gq")
        nc.sync.dma_start(gq_sb[:], gq_d[:])
        bq_sb = constp.tile([e, 1], F32, tag="bq")
        nc.sync.dma_start(bq_sb[:], bq_d[:])
        gk_sb = constp.tile([e, 1], F32, tag="gk")
        nc.sync.dma_start(gk_sb[:], gk_d[:])
        bk_sb = constp.tile([e, 1], F32, tag="bk")
        nc.sync.dma_start(bk_sb[:], bk_d[:])
        bhg_sb = constp.tile([P, nct], F32, tag="bhg")
        nc.sync.dma_start(bhg_sb[:], bhgT_d[:])
        bhv_sb = constp.tile([1, hh], F32, tag="bhv")
        nc.sync.dma_start(bhv_sb[:], bhv_d[:])
        ones_sb = constp.tile([1, P], F32, tag="ones")
        nc.vector.memset(ones_sb[:], 1.0)

        # tiny AllGather to warm the collective stream (first real op
        # otherwise pays a ~50us cold-start)
        warm_in = dramp.tile([1, 64], F32, tag="warm_in", name="warm_in")
        warm_out = dramp.tile([2, 64], F32, tag="warm_out", name="warm_out")
        warm_sb = constp.tile([1, 64], F32, tag="warm_sb")
        nc.vector.memset(warm_sb[:], 0.0)
        nc.gpsimd.dma_start(warm_in[:], warm_sb[:])
        nc.gpsimd.collective_compute("AllGather", ALU.bypass,
                                     replica_groups=pairs,
                                     ins=[warm_in.opt()],
                                     outs=[warm_out.opt()])

        # persistent activations
        qT_sb = mainp.tile([e, nmy], BF16, tag="qT", name="qT")
        kT_sb = mainp.tile([e, seq], BF16, tag="kT", name="kT")
        v8_sb = mainp.tile([P, njt, H], F8, tag="v8", name="v8")
        gate8_sb = mainp.tile([P, nct, nmy], F8, tag="gate8", name="gate8")
        at8_sb = mainp.tile([P, njt, IC], F8, tag="at8", name="at8")

        def scores(c, pool):
            # attnT[j, own chunk c] = relu(sim*sqrt(sa)/seq)^2 -> fp8
            chunk = slice(c * IC, (c + 1) * IC)
            for jt in range(njt):
                ps = psump.tile([P, IC], F32, tag="ps", name="ps")
                nc.tensor.matmul(ps[:], kT_sb[:, jt * P:(jt + 1) * P],
                                 qT_sb[:, chunk], start=True, stop=True)
                rstage = pool.tile([P, IC], BF16, tag="rstage", bufs=2,
                                   name="rstage")
                nc.scalar.activation(rstage[:], ps[:], AF.Relu, scale=RSC)
                nc.vector.tensor_tensor(at8_sb[:, jt, :], rstage[:],
                                        rstage[:], ALU.mult)

        with tc.tile_pool(name="ph1", bufs=1) as ph1p, \
             tc.tile_pool(name="wstream", bufs=1) as wsp, \
             tc.tile_pool(name="qkp", bufs=1) as qkp:
            x8g_sb = ph1p.tile([P, nd2, 2, seq], F8, tag="x8g", name="x8g")
            for quar in range(4):
                cols = slice(quar * (seq // 4), (quar + 1) * (seq // 4))
                nc.sync.dma_start(x8g_sb[:, :, :, cols], x8g_d[:, :, :, cols])
            x8m_sb = ph1p.tile([P, nd2, 2, nmy], F8, tag="x8m", name="x8m")

            # ---- qk projection (fp8 DoubleRow) ----
            # kT from the global-order x, qT from the own-rows x.
            # silu(u) = u * sigmoid(u); no Silu LUT in the interp, so build
            # from Sigmoid (ACT) + mult (DVE).
            qkg_sb = qkp.tile([e, seq], BF16, tag="qkg", name="qkg")
            qkm_sb = qkp.tile([e, nmy], BF16, tag="qkm", name="qkm")

            def qk_proj(n_chunks, xsb, qk_sb):
                for ic in range(n_chunks):
                    cols = slice(ic * IC, (ic + 1) * IC)
                    ps = psump.tile([e, IC], F32, tag="ps", name="ps")
                    for dp in range(nd2):
                        nc.tensor.matmul(ps[:], wqk8_sb[:, dp],
                                         xsb[:, dp, :, cols],
                                         start=(dp == 0),
                                         stop=(dp == nd2 - 1),
                                         perf_mode=DR)
                    nc.scalar.activation(qk_sb[:, cols], ps[:], AF.Silu,
                                         bias=bqk_sb[:], scale=INV_H)

            wv8 = [wsp.tile([P, nd2, 2, CC], F8, tag=f"wv8{i}",
                            name=f"wv8{i}") for i in range(2)]
            nc.scalar.dma_start(wv8[0][:], whv8_d[0])
            nc.scalar.dma_start(wv8[1][:], whv8_d[1])
            nc.scalar.dma_start(x8m_sb[:], x8my_d[:])

            qk_proj(n_ic, x8g_sb, qkg_sb)
            nc.vector.tensor_scalar(kT_sb[:], qkg_sb[:], gk_sb[:],
                                    bk_sb[:], ALU.mult, ALU.add)

            # ---- v half: v[j, c], exchanged via pairwise AllGather ----
            # the AG output is indexed by source rank = hidden-half index,
            # so assembling the full v is rank-independent; the per-slab
            # pipeline hides the exchange behind the gate projection below
            for cc in range(n_cc):
                ccs = slice(cc * CC, (cc + 1) * CC)
                wv = wv8[cc % 2]
                for jt in range(njt):
                    jts = slice(jt * P, (jt + 1) * P)
                    ps = psump.tile([P, CC], F32, tag="ps", name="ps")
                    for dp in range(nd2):
                        nc.tensor.matmul(ps[:], x8g_sb[:, dp, :, jts],
                                         wv[:, dp], start=(dp == 0),
                                         stop=(not with_bhv and dp == nd2 - 1),
                                         perf_mode=DR)
                    if with_bhv:
                        nc.tensor.matmul(ps[:], ones_sb[:], bhv_sb[:, ccs],
                                         start=False, stop=True)
                    vs = wsp.tile([P, CC], BF16, tag="vs", bufs=2, name="vs")
                    nc.scalar.activation(vs[:], ps[:], AF.Silu, scale=INV_H)
                    vst = wsp.tile([P, CC], F8, tag="vst", bufs=3, name="vst")
                    nc.vector.tensor_scalar(vst[:], vs[:], SV, None, ALU.mult)
                    nc.scalar.dma_start(vgi[cc][:, jt, :], vst[:])
                nc.gpsimd.collective_compute("AllGather", ALU.bypass,
                                             replica_groups=pairs,
                                             ins=[vgi[cc].opt()],
                                             outs=[vgo[cc].opt()])
                for s in range(2):
                    c0 = s * hh + cc * CC
                    nc.gpsimd.dma_start(v8_sb[:, :, c0:c0 + CC], vgo[cc][s])
                if cc + 2 < n_cc:
                    nc.scalar.dma_start(wv[:], whv8_d[cc + 2])

            wg8 = [wsp.tile([P, nd2, 2, P], F8, tag=f"wg8{i}",
                            name=f"wg8{i}") for i in range(4)]
            for i in range(4):
                nc.sync.dma_start(wg8[i][:], whg8_d[i])

            qk_proj(n_mc, x8m_sb, qkm_sb)
            nc.vector.tensor_scalar(qT_sb[:], qkm_sb[:], gq_sb[:],
                                    bq_sb[:], ALU.mult, ALU.add)
            # scores for own chunk 0: the relu/square hide under the gate
            # projection's matmuls
            scores(0, wsp)

            # ---- gate: gateT[c, i] over the FULL hidden, own rows only ----
            for ct in range(nct):
                cts = slice(ct * P, (ct + 1) * P)
                wg = wg8[ct % 4]
                for ic in range(n_mc):
                    cols = slice(ic * IC, (ic + 1) * IC)
                    ps = psump.tile([P, IC], F32, tag="ps", name="ps")
                    for dp in range(nd2):
                        nc.tensor.matmul(ps[:], wg[:, dp],
                                         x8m_sb[:, dp, :, cols],
                                         start=(dp == 0), stop=(dp == nd2 - 1),
                                         perf_mode=DR)
                    gs = wsp.tile([P, IC], BF16, tag="gs", bufs=2, name="gs")
                    nc.scalar.activation(gs[:], ps[:], AF.Silu,
                                         bias=bhg_sb[:, ct:ct + 1],
                                         scale=INV_H)
                    nc.vector.tensor_scalar(gate8_sb[:, ct, cols], gs[:],
                                            KGF, None, ALU.mult)
                if ct + 4 < nct:
                    nc.sync.dma_start(wg[:], whg8_d[ct + 4])

        # ---- attention + output over own rows ----
        with tc.tile_pool(name="ph2", bufs=1) as ph2p:
            og8_sb = ph2p.tile([P, nct, nmy], F8, tag="og8", name="og8")
            # prefetch the first two W_out slabs during the og matmuls
            wo8 = [ph2p.tile([P, ncp, 2, DC], F8, tag=f"wo8{i}",
                             name=f"wo8{i}") for i in range(2)]
            nc.sync.dma_start(wo8[0][:], wout8_d[0])
            nc.sync.dma_start(wo8[1][:], wout8_d[1])

            def og(c):
                # ogT[c over FULL hidden, own chunk] = (v^T attnT) * gate
                chunk = slice(c * IC, (c + 1) * IC)
                for ct in range(nct):
                    cts = slice(ct * P, (ct + 1) * P)
                    ps = psump.tile([P, IC], F32, tag="ps", name="ps")
                    for jp in range(njp):
                        nc.tensor.matmul(ps[:],
                                         v8_sb[:, 2 * jp:2 * jp + 2, cts],
                                         at8_sb[:, 2 * jp:2 * jp + 2, :],
                                         start=(jp == 0), stop=(jp == njp - 1),
                                         perf_mode=DR)
                    t2 = ph2p.tile([P, IC], BF16, tag="t2", bufs=2, name="t2")
                    nc.vector.tensor_scalar(t2[:], ps[:], OG_SC, None,
                                            ALU.mult)
                    nc.vector.tensor_tensor(og8_sb[:, ct, chunk], t2[:],
                                            gate8_sb[:, ct, chunk], ALU.mult)

            og(0)
            scores(1, ph2p)   # relu/square hide under og(0)'s matmuls
            og(1)
            # branch[own rows, :] = ogT^T @ Wout (full 4096 contraction),
            # fp8 W_out streamed per 512-col output slab, double-buffered
            # two slabs ahead, straight to the output
            for dc in range(n_dc):
                dcs = slice(dc * DC, (dc + 1) * DC)
                w = wo8[dc % 2]
                for it in range(nmy // P):
                    its = slice(it * P, (it + 1) * P)
                    ps = psump.tile([P, DC], F32, tag="ps", name="ps")
                    for cp in range(ncp):
                        nc.tensor.matmul(ps[:],
                                         og8_sb[:, 2 * cp:2 * cp + 2, its],
                                         w[:, cp, :, :],
                                         start=(cp == 0),
                                         stop=(cp == ncp - 1),
                                         perf_mode=DR)
                    po = ph2p.tile([P, DC], BF16, tag="po", bufs=4, name="po")
                    nc.vector.tensor_scalar(po[:], ps[:], INV_O, None,
                                            ALU.mult)
                    nc.scalar.dma_start(out_d[it * P:(it + 1) * P, dcs],
                                        po[:])
                if dc + 2 < n_dc:
                    nc.sync.dma_start(w[:], wout8_d[dc + 2])

    nc.compile()
    return nc


def own_rows(seq, h):
    """Rows owned by pair-member h: contiguous half of the sequence."""
    nmy = seq // 2
    return np.arange(h * nmy, (h + 1) * nmy)


def _q8(a, s):
    """Quantize a*s to fp8 e4m3, clipping to +-240 (TRN e4m3 infs at 256)."""
    return np.clip(a.astype(np.float32) * s, -F8MAX, F8MAX).astype(
        ml_dtypes.float8_e4m3)


def _dr_pack(m8):
    """[K, F] fp8 -> [128, K//256, 2, F] DoubleRow pair layout."""
    K, F = m8.shape
    return np.ascontiguousarray(
        m8.reshape(K // 256, 2, P, F).transpose(2, 0, 1, 3))


def make_in_maps(x, W_hidden, b_hidden, W_qk, b_qk, gamma_q, beta_q,
                 gamma_k, beta_k, W_out, b_out, n_cores=8):
    """Host-side quantization/layout prep.  Returns per-core input dicts."""
    B, seq, dim = x.shape
    H2 = W_hidden.shape[1]
    H = H2 // 2
    hh = H // 2
    nct = H // P
    nmy = seq // 2
    f32 = np.float32
    x8_cache = {}
    xm_cache = {}
    half_cache = {}
    # full-hidden tensors, shared by both pair members
    _wg = _dr_pack(_q8(W_hidden[:, H:], SW))
    whg8 = np.ascontiguousarray(np.stack(
        [_wg[:, :, :, c0:c0 + P] for c0 in range(0, H, P)]))
    _wo = _dr_pack(_q8(W_out, SWO))
    wout8 = np.ascontiguousarray(np.stack(
        [_wo[:, :, :, dc * 512:(dc + 1) * 512] for dc in range(dim // 512)]))
    wqk8 = _dr_pack(_q8(W_qk, SW))
    bhgT = np.ascontiguousarray(b_hidden[H:].astype(f32).reshape(nct, P).T)

    def halves(h):
        if h not in half_cache:
            cs = slice(h * hh, (h + 1) * hh)
            half_cache[h] = {
                "whv8": np.ascontiguousarray(np.stack(
                    [_dr_pack(_q8(W_hidden[:, cs], SW))[:, :, :, c0:c0 + 512]
                     for c0 in range(0, hh, 512)])),
                "bhv": (b_hidden[cs].astype(f32) * (SX * SW)).reshape(1, -1),
            }
        return half_cache[h]

    in_maps = []
    for core in range(n_cores):
        b, h = core // 2, core % 2
        if b not in x8_cache:
            x8_cache[b] = _dr_pack(_q8(np.ascontiguousarray(x[b].T), SX))
        if (b, h) not in xm_cache:
            xm_cache[(b, h)] = np.ascontiguousarray(
                x8_cache[b][:, :, :, h * nmy:(h + 1) * nmy])
        hv = halves(h)
        in_maps.append({
            "x8g": x8_cache[b],
            "x8my": xm_cache[(b, h)],
            "whv8": hv["whv8"],
            "whg8": whg8,
            "wout8": wout8,
            "wqk8": wqk8,
            "bqk": b_qk.reshape(-1, 1).astype(f32),
            "bqks": (b_qk.reshape(-1, 1).astype(f32) * (SX * SW)),
            "gq": gamma_q.reshape(-1, 1).astype(f32),
            "bq": beta_q.reshape(-1, 1).astype(f32),
            "gk": gamma_k.reshape(-1, 1).astype(f32),
            "bk": beta_k.reshape(-1, 1).astype(f32),
            "bhv": hv["bhv"],
            "bhgT": bhgT,
            "bhgTs": bhgT * (SX * SW),
        })
    return in_maps


_NC_CACHE = {}


def _get_nc(seq, dim, hh, n_cores, with_bhv=True):
    key = (seq, dim, hh, n_cores, with_bhv)
    if key not in _NC_CACHE:
        _NC_CACHE[key] = build_gau_nc(seq=seq, dim=dim, hh=hh,
                                      n_cores=n_cores, with_bhv=with_bhv)
    return _NC_CACHE[key]


def kernel(x, W_hidden, b_hidden, W_qk, b_qk, gamma_q, beta_q, gamma_k,
           beta_k, W_out, b_out):
    x = np.asarray(x)
    B, seq, dim = x.shape
    hh = W_hidden.shape[1] // 4
    n_cores = 2 * B
    with_bhv = bool(np.any(np.asarray(b_hidden)[: 2 * hh] != 0))
    nc = _get_nc(seq, dim, hh, n_cores, with_bhv=with_bhv)
    in_maps = make_in_maps(x, np.asarray(W_hidden), np.asarray(b_hidden),
                           np.asarray(W_qk), np.asarray(b_qk),
                           np.asarray(gamma_q), np.asarray(beta_q),
                           np.asarray(gamma_k), np.asarray(beta_k),
                           np.asarray(W_out), np.asarray(b_out),
                           n_cores=n_cores)
    res = run_bass_kernel_spmd(nc, in_maps, core_ids=list(range(n_cores)))
    bo = np.asarray(b_out).astype(np.float32)
    out = np.empty((B, seq, dim), np.float32)
    for b in range(B):
        for h in range(2):
            rows = own_rows(seq, h)
            out[b, rows] = (res.results[2 * b + h]["out"].astype(np.float32)
                            + x[b][rows] + bo)
    return out


# revision 22
# speedup vs baseline: 1.2173x; 1.0185x over previous
"""GAU (Gated Attention Unit, relu^2 attention) Trainium2 Bass kernel, fp8.

Problem shapes: x [4, 2048, 2048] f32; W_hidden [2048, 8192]; W_qk [2048, 128];
W_out [4096, 2048]; out = GAU(x) + x.

Sharding (8 cores): core = 2*batch + h, h in {0,1}.  Each pair of cores
handles one batch.  The VALUE/GATE hidden projections are column-split in
half within the pair (core h computes v for hidden cols
[h*2048:(h+1)*2048]); the computed v half is then exchanged with a
pairwise AllGather (fp8, 4MB, pipelined per 512-column slab behind the
gate projection) so both cores hold the full [2048, 4096] v.  An
AllGather works here precisely because both cores want the SAME full v
-- its output is indexed by source rank, which equals the hidden-half
index, keeping the program rank-independent.

Everything downstream is split by QUERY rows instead: core h owns the
global rows [h*1024, (h+1)*1024) and computes scores, attn, gate, og and
the output projection (full 4096-deep contraction, W_out streamed fp8)
only for those rows, writing final bf16 branch rows straight to its
output.  The row split is baked in on the host: a second input tensor
x8my carries only the core's own 1024 x columns for the q projection and
the gate's moving operand, while x8g (global order) feeds k, and v.
There is no partial-sum ReduceScatter and no serial collective tail.
The host adds the residual x and b_out in fp32 (the branch is ~0.3% of
the output norm, so a bf16 branch costs ~3e-6 relative error).

All projections run as fp8e4 (e4m3, max +-240) DoubleRow matmuls: both
operands are packed [128, 2, free] pairing two 128-row contraction
tiles, giving 2 fp8 MACs/PE-cell/cycle.  Quantization scales are powers
of two, chosen so every fp8 tensor's max is >=2.7x below 240 (measured
on the real data distribution; host clips before casting to avoid
e4m3's non-saturating inf at 256):

  x*16, W_hidden*512, W_qk*512, W_out*512, v*16, attn*2^14, gate*32,
  og*1024

PSUM stays fp32; descales fold into the activation `scale` ports and DVE
tensor_scalar immediates.  silu runs on the scalar engine's native Silu
LUT (one ACT op; the CoreSim interp lacks the LUT, so this kernel is
hardware-only).  qT/kT and the relu^2 scores matmul stay bf16 (K=128,
cheap; scores are NOT pair-replicated since each core only needs its own
rows).  The scores for chunk 0 are emitted between v and gate, and for
chunk 1 between og(0) and og(1), hiding their scalar-engine relu under
matmul-heavy phases.  W_out streams per 512-column output slab from a
slab-contiguous DRAM layout, double-buffered two slabs ahead.
"""

import numpy as np
import ml_dtypes
from contextlib import ExitStack

import concourse.bass as bass
import concourse.bacc as bacc
import concourse.mybir as mybir
import concourse.tile as tile
from concourse.bass_utils import run_bass_kernel_spmd

BF16 = mybir.dt.bfloat16
F32 = mybir.dt.float32
F8 = mybir.dt.float8e4
AF = mybir.ActivationFunctionType
ALU = mybir.AluOpType
DR = mybir.MatmulPerfMode.DoubleRow
P = 128
F8MAX = 240.0

# quantization scales (powers of two; see module docstring)
SX = 16.0       # x
SW = 512.0      # W_hidden (both halves), W_qk
SWO = 512.0     # W_out
SV = 16.0       # v activations
SA = 16384.0    # attn = relu(sim)^2    (sqrt(SA) = 128 folds into the relu)
KGF = 32.0      # gate activations
SO = 1024.0     # og activations

INV_H = 1.0 / (SX * SW)        # hidden/qk psum -> real pre-activation
T_V = SV / (SX * SW)           # v-path u multiplier
T_G = KGF / (SX * SW)          # gate-path u multiplier
OG_SC = SO / (SA * SV * KGF)   # og = (psum_av * OG_SC) * gate8
RSC = 128.0 / 2048.0           # sqrt(SA)/seq relu scale
INV_O = 1.0 / (SO * SWO)       # out psum -> real branch rows


def build_gau_nc(seq=2048, dim=2048, hh=2048, n_cores=8, with_bhv=True):
    """Build the SPMD program.  hh = per-core v half width."""
    e = P
    nd2 = dim // (2 * P)   # DoubleRow contraction pair tiles
    njt = seq // P
    njp = njt // 2
    H = 2 * hh             # full hidden width
    nct = H // P           # full-hidden c tiles (gate, og)
    ncp = nct // 2
    nmy = seq // 2         # own query rows per core
    IC = 512
    n_ic = seq // IC
    n_mc = nmy // IC       # own-row chunks
    CC = 512
    n_cc = hh // CC
    DC = 512
    n_dc = dim // DC
    pairs = [[2 * g, 2 * g + 1] for g in range(n_cores // 2)]

    nc = bacc.Bacc("TRN2", target_bir_lowering=False, debug=False,
                   num_devices=n_cores)

    x8g_d = nc.dram_tensor("x8g", [P, nd2, 2, seq], F8, kind="ExternalInput")
    x8my_d = nc.dram_tensor("x8my", [P, nd2, 2, nmy], F8,
                            kind="ExternalInput")
    whv8_d = nc.dram_tensor("whv8", [n_cc, P, nd2, 2, CC], F8,
                            kind="ExternalInput")
    whg8_d = nc.dram_tensor("whg8", [nct, P, nd2, 2, P], F8,
                            kind="ExternalInput")
    wqk8_d = nc.dram_tensor("wqk8", [P, nd2, 2, e], F8, kind="ExternalInput")
    wout8_d = nc.dram_tensor("wout8", [n_dc, P, ncp, 2, DC], F8,
                             kind="ExternalInput")
    bqk_d = nc.dram_tensor("bqk", [e, 1], F32, kind="ExternalInput")
    bqks_d = nc.dram_tensor("bqks", [e, 1], F32, kind="ExternalInput")
    gq_d = nc.dram_tensor("gq", [e, 1], F32, kind="ExternalInput")
    bq_d = nc.dram_tensor("bq", [e, 1], F32, kind="ExternalInput")
    gk_d = nc.dram_tensor("gk", [e, 1], F32, kind="ExternalInput")
    bk_d = nc.dram_tensor("bk", [e, 1], F32, kind="ExternalInput")
    bhv_d = nc.dram_tensor("bhv", [1, hh], F32, kind="ExternalInput")
    bhgT_d = nc.dram_tensor("bhgT", [P, nct], F32, kind="ExternalInput")
    bhgTs_d = nc.dram_tensor("bhgTs", [P, nct], F32, kind="ExternalInput")
    out_d = nc.dram_tensor("out", [nmy, dim], BF16, kind="ExternalOutput")

    with tile.TileContext(nc) as tc, ExitStack() as st:
        constp = st.enter_context(tc.tile_pool(name="const", bufs=1))
        psump = st.enter_context(tc.tile_pool(name="psum", bufs=8, space="PSUM"))
        dramp = st.enter_context(tc.tile_pool(name="dram", bufs=1, space="DRAM"))
        mainp = st.enter_context(tc.tile_pool(name="main", bufs=1))

        # v AllGather bounce buffers, one per 512-col slab of the own half
        vgi = [dramp.tile([P, njt, CC], F8, tag=f"vgi{k}", name=f"vgi{k}")
               for k in range(n_cc)]
        vgo = [dramp.tile([2, P, njt, CC], F8, tag=f"vgo{k}", name=f"vgo{k}")
               for k in range(n_cc)]

        # ---- constants ----
        wqk8_sb = constp.tile([P, nd2, 2, e], F8, tag="wqk8")
        nc.sync.dma_start(wqk8_sb[:], wqk8_d[:])
        bqk_sb = constp.tile([e, 1], F32, tag="bqk")
        nc.sync.dma_start(bqk_sb[:], bqk_d[:])
        gq_sb = constp.tile([e, 1], F32, tag="gq")
        nc.scalar.dma_start(gq_sb[:], gq_d[:])
        bq_sb = constp.tile([e, 1], F32, tag="bq")
        nc.scalar.dma_start(bq_sb[:], bq_d[:])
        gk_sb = constp.tile([e, 1], F32, tag="gk")
        nc.scalar.dma_start(gk_sb[:], gk_d[:])
        bk_sb = constp.tile([e, 1], F32, tag="bk")
        nc.scalar.dma_start(bk_sb[:], bk_d[:])
        bhg_sb = constp.tile([P, nct], F32, tag="bhg")
        nc.scalar.dma_start(bhg_sb[:], bhgT_d[:])
        if with_bhv:
            bhv_sb = constp.tile([1, hh], F32, tag="bhv")
            nc.scalar.dma_start(bhv_sb[:], bhv_d[:])
            ones_sb = constp.tile([1, P], F32, tag="ones")
            nc.vector.memset(ones_sb[:], 1.0)

        # tiny AllGather to warm the collective stream (first real op
        # otherwise pays a ~50us cold-start)
        warm_in = dramp.tile([1, 64], F32, tag="warm_in", name="warm_in")
        warm_out = dramp.tile([2, 64], F32, tag="warm_out", name="warm_out")
        warm_sb = constp.tile([1, 64], F32, tag="warm_sb")
        nc.vector.memset(warm_sb[:], 0.0)
        nc.gpsimd.dma_start(warm_in[:], warm_sb[:])
        nc.gpsimd.collective_compute("AllGather", ALU.bypass,
                                     replica_groups=pairs,
                                     ins=[warm_in.opt()],
                                     outs=[warm_out.opt()])

        # persistent activations
        qT_sb = mainp.tile([e, nmy], BF16, tag="qT", name="qT")
        kT_sb = mainp.tile([e, seq], BF16, tag="kT", name="kT")
        v8_sb = mainp.tile([P, njt, H], F8, tag="v8", name="v8")
        gate8_sb = mainp.tile([P, nct, nmy], F8, tag="gate8", name="gate8")
        at8_sb = mainp.tile([P, njt, IC], F8, tag="at8", name="at8")

        def scores(c, pool):
            # attnT[j, own chunk c] = relu(sim*sqrt(sa)/seq)^2 -> fp8
            chunk = slice(c * IC, (c + 1) * IC)
            for jt in range(njt):
                ps = psump.tile([P, IC], F32, tag="ps", name="ps")
                nc.tensor.matmul(ps[:], kT_sb[:, jt * P:(jt + 1) * P],
                                 qT_sb[:, chunk], start=True, stop=True)
                rstage = pool.tile([P, IC], BF16, tag="rstage", bufs=2,
                                   name="rstage")
                nc.scalar.activation(rstage[:], ps[:], AF.Relu, scale=RSC)
                nc.vector.tensor_tensor(at8_sb[:, jt, :], rstage[:],
                                        rstage[:], ALU.mult)

        with tc.tile_pool(name="ph1", bufs=1) as ph1p, \
             tc.tile_pool(name="wstream", bufs=1) as wsp, \
             tc.tile_pool(name="qkp", bufs=1) as qkp:
            x8g_sb = ph1p.tile([P, nd2, 2, seq], F8, tag="x8g", name="x8g")
            for quar in range(4):
                cols = slice(quar * (seq // 4), (quar + 1) * (seq // 4))
                nc.sync.dma_start(x8g_sb[:, :, :, cols], x8g_d[:, :, :, cols])
            x8m_sb = ph1p.tile([P, nd2, 2, nmy], F8, tag="x8m", name="x8m")

            # ---- qk projection (fp8 DoubleRow) ----
            # kT from the global-order x, qT from the own-rows x.
            # silu(u) = u * sigmoid(u); no Silu LUT in the interp, so build
            # from Sigmoid (ACT) + mult (DVE).
            qkg_sb = qkp.tile([e, seq], BF16, tag="qkg", name="qkg")
            qkm_sb = qkp.tile([e, nmy], BF16, tag="qkm", name="qkm")

            def qk_proj(n_chunks, xsb, qk_sb):
                for ic in range(n_chunks):
                    cols = slice(ic * IC, (ic + 1) * IC)
                    ps = psump.tile([e, IC], F32, tag="ps", name="ps")
                    for dp in range(nd2):
                        nc.tensor.matmul(ps[:], wqk8_sb[:, dp],
                                         xsb[:, dp, :, cols],
                                         start=(dp == 0),
                                         stop=(dp == nd2 - 1),
                                         perf_mode=DR)
                    nc.scalar.activation(qk_sb[:, cols], ps[:], AF.Silu,
                                         bias=bqk_sb[:], scale=INV_H)

            wv8 = [wsp.tile([P, nd2, 2, CC], F8, tag=f"wv8{i}",
                            name=f"wv8{i}") for i in range(2)]
            nc.scalar.dma_start(wv8[0][:], whv8_d[0])
            nc.scalar.dma_start(wv8[1][:], whv8_d[1])
            nc.sync.dma_start(x8m_sb[:], x8my_d[:])

            qk_proj(n_ic, x8g_sb, qkg_sb)
            nc.vector.tensor_scalar(kT_sb[:], qkg_sb[:], gk_sb[:],
                                    bk_sb[:], ALU.mult, ALU.add)

            # ---- v half: v[j, c], exchanged via pairwise AllGather ----
            # the AG output is indexed by source rank = hidden-half index,
            # so assembling the full v is rank-independent; the per-slab
            # pipeline hides the exchange behind the gate projection below
            for cc in range(n_cc):
                ccs = slice(cc * CC, (cc + 1) * CC)
                wv = wv8[cc % 2]
                for jt in range(njt):
                    jts = slice(jt * P, (jt + 1) * P)
                    ps = psump.tile([P, CC], F32, tag="ps", name="ps")
                    for dp in range(nd2):
                        nc.tensor.matmul(ps[:], x8g_sb[:, dp, :, jts],
                                         wv[:, dp], start=(dp == 0),
                                         stop=(not with_bhv and dp == nd2 - 1),
                                         perf_mode=DR)
                    if with_bhv:
                        nc.tensor.matmul(ps[:], ones_sb[:], bhv_sb[:, ccs],
                                         start=False, stop=True)
                    vs = wsp.tile([P, CC], BF16, tag="vs", bufs=2, name="vs")
                    nc.scalar.activation(vs[:], ps[:], AF.Silu, scale=INV_H)
                    vst = wsp.tile([P, CC], F8, tag="vst", bufs=3, name="vst")
                    nc.vector.tensor_scalar(vst[:], vs[:], SV, None, ALU.mult)
                    nc.scalar.dma_start(vgi[cc][:, jt, :], vst[:])
                nc.gpsimd.collective_compute("AllGather", ALU.bypass,
                                             replica_groups=pairs,
                                             ins=[vgi[cc].opt()],
                                             outs=[vgo[cc].opt()])
                for s in range(2):
                    c0 = s * hh + cc * CC
                    nc.gpsimd.dma_start(v8_sb[:, :, c0:c0 + CC], vgo[cc][s])
                if cc + 2 < n_cc:
                    nc.scalar.dma_start(wv[:], whv8_d[cc + 2])

            wg8 = [wsp.tile([P, nd2, 2, P], F8, tag=f"wg8{i}",
                            name=f"wg8{i}") for i in range(4)]
            for i in range(4):
                nc.sync.dma_start(wg8[i][:], whg8_d[i])

            qk_proj(n_mc, x8m_sb, qkm_sb)
            nc.vector.tensor_scalar(qT_sb[:], qkm_sb[:], gq_sb[:],
                                    bq_sb[:], ALU.mult, ALU.add)
            # scores for own chunk 0: the relu/square hide under the gate
            # projection's matmuls
            scores(0, wsp)

            # ---- gate: gateT[c, i] over the FULL hidden, own rows only ----
            for ct in range(nct):
                cts = slice(ct * P, (ct + 1) * P)
                wg = wg8[ct % 4]
                for ic in range(n_mc):
                    cols = slice(ic * IC, (ic + 1) * IC)
                    ps = psump.tile([P, IC], F32, tag="ps", name="ps")
                    for dp in range(nd2):
                        nc.tensor.matmul(ps[:], wg[:, dp],
                                         x8m_sb[:, dp, :, cols],
                                         start=(dp == 0), stop=(dp == nd2 - 1),
                                         perf_mode=DR)
                    gs = wsp.tile([P, IC], BF16, tag="gs", bufs=2, name="gs")
                    nc.scalar.activation(gs[:], ps[:], AF.Silu,
                                         bias=bhg_sb[:, ct:ct + 1],
                                         scale=INV_H)
                    nc.vector.tensor_scalar(gate8_sb[:, ct, cols], gs[:],
                                            KGF, None, ALU.mult)
                if ct + 4 < nct:
                    nc.sync.dma_start(wg[:], whg8_d[ct + 4])

        # ---- attention + output over own rows ----
        with tc.tile_pool(name="ph2", bufs=1) as ph2p:
            og8_sb = ph2p.tile([P, nct, nmy], F8, tag="og8", name="og8")
            # prefetch the first two W_out slabs during the og matmuls
            wo8 = [ph2p.tile([P, ncp, 2, DC], F8, tag=f"wo8{i}",
                             name=f"wo8{i}") for i in range(2)]
            nc.sync.dma_start(wo8[0][:], wout8_d[0])
            nc.sync.dma_start(wo8[1][:], wout8_d[1])

            def og(c):
                # ogT[c over FULL hidden, own chunk] = (v^T attnT) * gate
                chunk = slice(c * IC, (c + 1) * IC)
                for ct in range(nct):
                    cts = slice(ct * P, (ct + 1) * P)
                    ps = psump.tile([P, IC], F32, tag="ps", name="ps")
                    for jp in range(njp):
                        nc.tensor.matmul(ps[:],
                                         v8_sb[:, 2 * jp:2 * jp + 2, cts],
                                         at8_sb[:, 2 * jp:2 * jp + 2, :],
                                         start=(jp == 0), stop=(jp == njp - 1),
                                         perf_mode=DR)
                    t2 = ph2p.tile([P, IC], BF16, tag="t2", bufs=2, name="t2")
                    nc.vector.tensor_scalar(t2[:], ps[:], OG_SC, None,
                                            ALU.mult)
                    nc.vector.tensor_tensor(og8_sb[:, ct, chunk], t2[:],
                                            gate8_sb[:, ct, chunk], ALU.mult)

            og(0)
            scores(1, ph2p)   # relu/square hide under og(0)'s matmuls
            og(1)
            # branch[own rows, :] = ogT^T @ Wout (full 4096 contraction),
            # fp8 W_out streamed per 512-col output slab, double-buffered
            # two slabs ahead, straight to the output
            for dc in range(n_dc):
                dcs = slice(dc * DC, (dc + 1) * DC)
                w = wo8[dc % 2]
                for it in range(nmy // P):
                    its = slice(it * P, (it + 1) * P)
                    ps = psump.tile([P, DC], F32, tag="ps", name="ps")
                    for cp in range(ncp):
                        nc.tensor.matmul(ps[:],
                                         og8_sb[:, 2 * cp:2 * cp + 2, its],
                                         w[:, cp, :, :],
                                         start=(cp == 0),
                                         stop=(cp == ncp - 1),
                                         perf_mode=DR)
                    po = ph2p.tile([P, DC], BF16, tag="po", bufs=4, name="po")
                    nc.vector.tensor_scalar(po[:], ps[:], INV_O, None,
                                            ALU.mult)
                    nc.scalar.dma_start(out_d[it * P:(it + 1) * P, dcs],
                                        po[:])
                if dc + 2 < n_dc:
                    nc.sync.dma_start(w[:], wout8_d[dc + 2])

    nc.compile()
    return nc


def own_rows(seq, h):
    """Rows owned by pair-member h: contiguous half of the sequence."""
    nmy = seq // 2
    return np.arange(h * nmy, (h + 1) * nmy)


def _q8(a, s):
    """Quantize a*s to fp8 e4m3, clipping to +-240 (TRN e4m3 infs at 256)."""
    return np.clip(a.astype(np.float32) * s, -F8MAX, F8MAX).astype(
        ml_dtypes.float8_e4m3)


def _dr_pack(m8):
    """[K, F] fp8 -> [128, K//256, 2, F] DoubleRow pair layout."""
    K, F = m8.shape
    return np.ascontiguousarray(
        m8.reshape(K // 256, 2, P, F).transpose(2, 0, 1, 3))


def make_in_maps(x, W_hidden, b_hidden, W_qk, b_qk, gamma_q, beta_q,
                 gamma_k, beta_k, W_out, b_out, n_cores=8):
    """Host-side quantization/layout prep.  Returns per-core input dicts."""
    B, seq, dim = x.shape
    H2 = W_hidden.shape[1]
    H = H2 // 2
    hh = H // 2
    nct = H // P
    nmy = seq // 2
    f32 = np.float32
    x8_cache = {}
    xm_cache = {}
    half_cache = {}
    # full-hidden tensors, shared by both pair members
    _wg = _dr_pack(_q8(W_hidden[:, H:], SW))
    whg8 = np.ascontiguousarray(np.stack(
        [_wg[:, :, :, c0:c0 + P] for c0 in range(0, H, P)]))
    _wo = _dr_pack(_q8(W_out, SWO))
    wout8 = np.ascontiguousarray(np.stack(
        [_wo[:, :, :, dc * 512:(dc + 1) * 512] for dc in range(dim // 512)]))
    wqk8 = _dr_pack(_q8(W_qk, SW))
    bhgT = np.ascontiguousarray(b_hidden[H:].astype(f32).reshape(nct, P).T)

    def halves(h):
        if h not in half_cache:
            cs = slice(h * hh, (h + 1) * hh)
            half_cache[h] = {
                "whv8": np.ascontiguousarray(np.stack(
                    [_dr_pack(_q8(W_hidden[:, cs], SW))[:, :, :, c0:c0 + 512]
                     for c0 in range(0, hh, 512)])),
                "bhv": (b_hidden[cs].astype(f32) * (SX * SW)).reshape(1, -1),
            }
        return half_cache[h]

    in_maps = []
    for core in range(n_cores):
        b, h = core // 2, core % 2
        if b not in x8_cache:
            x8_cache[b] = _dr_pack(_q8(np.ascontiguousarray(x[b].T), SX))
        if (b, h) not in xm_cache:
            xm_cache[(b, h)] = np.ascontiguousarray(
                x8_cache[b][:, :, :, h * nmy:(h + 1) * nmy])
        hv = halves(h)
        in_maps.append({
            "x8g": x8_cache[b],
            "x8my": xm_cache[(b, h)],
            "whv8": hv["whv8"],
            "whg8": whg8,
            "wout8": wout8,
            "wqk8": wqk8,
            "bqk": b_qk.reshape(-1, 1).astype(f32),
            "bqks": (b_qk.reshape(-1, 1).astype(f32) * (SX * SW)),
            "gq": gamma_q.reshape(-1, 1).astype(f32),
            "bq": beta_q.reshape(-1, 1).astype(f32),
            "gk": gamma_k.reshape(-1, 1).astype(f32),
            "bk": beta_k.reshape(-1, 1).astype(f32),
            "bhv": hv["bhv"],
            "bhgT": bhgT,
            "bhgTs": bhgT * (SX * SW),
        })
    return in_maps


_NC_CACHE = {}


def _get_nc(seq, dim, hh, n_cores, with_bhv=True):
    key = (seq, dim, hh, n_cores, with_bhv)
    if key not in _NC_CACHE:
        _NC_CACHE[key] = build_gau_nc(seq=seq, dim=dim, hh=hh,
                                      n_cores=n_cores, with_bhv=with_bhv)
    return _NC_CACHE[key]


def kernel(x, W_hidden, b_hidden, W_qk, b_qk, gamma_q, beta_q, gamma_k,
           beta_k, W_out, b_out):
    x = np.asarray(x)
    B, seq, dim = x.shape
    hh = W_hidden.shape[1] // 4
    n_cores = 2 * B
    with_bhv = bool(np.any(np.asarray(b_hidden)[: 2 * hh] != 0))
    nc = _get_nc(seq, dim, hh, n_cores, with_bhv=with_bhv)
    in_maps = make_in_maps(x, np.asarray(W_hidden), np.asarray(b_hidden),
                           np.asarray(W_qk), np.asarray(b_qk),
                           np.asarray(gamma_q), np.asarray(beta_q),
                           np.asarray(gamma_k), np.asarray(beta_k),
                           np.asarray(W_out), np.asarray(b_out),
                           n_cores=n_cores)
    res = run_bass_kernel_spmd(nc, in_maps, core_ids=list(range(n_cores)))
    bo = np.asarray(b_out).astype(np.float32)
    out = np.empty((B, seq, dim), np.float32)
    for b in range(B):
        for h in range(2):
            rows = own_rows(seq, h)
            out[b, rows] = (res.results[2 * b + h]["out"].astype(np.float32)
                            + x[b][rows] + bo)
    return out
